# revision 1
# baseline (speedup 1.0000x reference)
"""MLA-style attention (nn_Attention_7868380086611) on 8 TRN2 NeuronCores.

Strategy
--------
The reference "absorbs" the up-projections (k_eff = Wuq_h @ Wuk_h per head,
v_eff = (W_uv.T @ W_o.T) per-head slices), which is ~4x more FLOPs than the
factored form.  By matmul associativity we instead compute standard per-head
q/k (head dim 128) plus the decoupled-RoPE part, and an effective per-head
v~_h = c_kv @ (W_uv.T @ W_o.T)[:, cols_h], so the [T,T] attention matrix only
ever multiplies 128-wide tensors.

Sharding: head-parallel attention (2 of 16 heads per core) on top of
T-sharded down-projections.  Each core computes c_q/c_kv/k_r for its T/8
token slice (transposed layout, contraction dims on partitions), then one
AllGather (~1 MB/rank, bf16) replicates the tiny latents, and each core runs
the full causal attention for its 2 heads, writing its own 256 output
columns.  All inputs are pre-cast/pre-tiled to bf16 on the host; PSUM
accumulation is fp32.

The same SPMD graph runs on all 8 cores; all rank-dependence is carried by
the per-core input slices.
"""

import math
import sys

import numpy as np

sys.path.insert(0, "/opt/trn_rl_repo")

import ml_dtypes  # noqa: E402

from concourse import bacc, bass, masks, mybir  # noqa: E402
from concourse.bass_utils import run_bass_kernel_spmd  # noqa: E402
from concourse.tile import TileContext  # noqa: E402

B, T, C = 1, 2048, 2048
NH, HS = 16, 128
NLQ, NLKV, DHR = 1536, 512, 64
NCORES = 8
HPC = NH // NCORES          # heads per core = 2
TS = T // NCORES            # 256-token shard for down-projections
P = 128
LQ = NLQ // P               # 12 l-chunks
LKV = NLKV // P             # 4
CCH = C // P                # 16 c-chunks
TJ = T // 512               # 4 t-chunks of 512
SC = T // P                 # 16 s-chunks
SCALE = 1.0 / math.sqrt(HS + DHR)
NEG = -1.0e10

BF = mybir.dt.bfloat16
F32 = mybir.dt.float32
Exp = mybir.ActivationFunctionType.Exp
Copy = mybir.ActivationFunctionType.Copy

GROUP = NLQ + NLKV + DHR    # 2112 rows in the all-gather buffer


def build_nc():
    nc = bacc.Bacc(None, target_bir_lowering=False, num_devices=NCORES)

    xT_sh = nc.declare_dram_parameter("xT_sh", [C, TS], BF, isOutput=False)
    wdqT = nc.declare_dram_parameter("wdqT", [LQ // 4, C, 512], BF, isOutput=False)
    wdkvT = nc.declare_dram_parameter("wdkvT", [1, C, 512], BF, isOutput=False)
    wkrT = nc.declare_dram_parameter("wkrT", [C, DHR], BF, isOutput=False)
    cos2T = nc.declare_dram_parameter("cos2T", [DHR, T], BF, isOutput=False)
    sin2T = nc.declare_dram_parameter("sin2T", [DHR, T], BF, isOutput=False)
    wuq = nc.declare_dram_parameter("wuq", [LQ, P, HPC * HS], BF, isOutput=False)
    wqrT = nc.declare_dram_parameter("wqrT", [LQ, P, HPC * DHR], BF, isOutput=False)
    wukT = nc.declare_dram_parameter("wukT", [LKV, P, HPC * HS], BF, isOutput=False)
    wuv = nc.declare_dram_parameter("wuv", [CCH, P, NLKV], BF, isOutput=False)
    woT = nc.declare_dram_parameter("woT", [CCH, P, HPC * HS], BF, isOutput=False)
    out = nc.declare_dram_parameter("out", [HPC * T, HS], F32, isOutput=True)

    GKV = NLKV + DHR
    cc_in_kv = nc.dram_tensor("cc_in_kv", [GKV, TS], BF)
    cc_out_kv = nc.dram_tensor("cc_out_kv", [NCORES, GKV, TS], BF,
                               addr_space="Shared")
    NQA = 8 * P          # l-chunks 0-7 in the first q gather
    cc_in_qa = nc.dram_tensor("cc_in_qa", [NQA, TS], BF)
    cc_out_qa = nc.dram_tensor("cc_out_qa", [NCORES, NQA, TS], BF,
                               addr_space="Shared")
    cc_in_qb = nc.dram_tensor("cc_in_qb", [NLQ - NQA, TS], BF)
    cc_out_qb = nc.dram_tensor("cc_out_qb", [NCORES, NLQ - NQA, TS], BF,
                               addr_space="Shared")

    with TileContext(nc) as tc:
        with (
            tc.tile_pool(name="persist", bufs=1) as persist,
            tc.tile_pool(name="lat", bufs=1) as lat,
            tc.tile_pool(name="proj", bufs=1) as proj,
            tc.tile_pool(name="wts", bufs=1) as wts,
        ):
            # ---- constants ----
            id_bf = persist.tile([P, P], BF)
            masks.make_identity(nc, id_bf[:])
            id_f32 = persist.tile([P, P], F32)
            masks.make_identity(nc, id_f32[:])
            ones_bf = persist.tile([P, 1], BF)
            nc.vector.memset(ones_bf[:], 1.0)
            # 4 additive causal masks [128, 512]: keep (0) iff t - s - 128*m >= 0
            cmask = persist.tile([P, 4 * 512], F32)
            nc.gpsimd.memset(cmask[:], 0.0)
            for m in range(4):
                nc.gpsimd.affine_select(
                    out=cmask[:, m * 512:(m + 1) * 512],
                    in_=cmask[:, m * 512:(m + 1) * 512],
                    compare_op=mybir.AluOpType.is_ge,
                    fill=NEG,
                    base=-m * P,
                    channel_multiplier=-1,
                    pattern=[[1, 512]],
                )
            cos_sb = persist.tile([DHR, T], BF)
            nc.scalar.dma_start(cos_sb[:], cos2T[:, :])
            sin_sb = persist.tile([DHR, T], BF)
            nc.scalar.dma_start(sin_sb[:], sin2T[:, :])

            # ---- phase 1: c_kv^T/k_r^T shard -> AG-kv first (small mesh,
            # early trigger), then c_q^T shard -> AG-q.  The kv-side
            # projections (k, v~) then run inside AG-q's mesh window.
            with (
                tc.tile_pool(name="p1w", bufs=2) as p1w,
                tc.tile_pool(name="p1ps", bufs=2, space="PSUM") as p1ps,
                tc.tile_pool(name="p1sh", bufs=3) as p1sh,
            ):
                xt = []
                for g in range(4):
                    t = lat.tile([P, 4 * TS], BF, name=f"xt{g}", tag=f"xt{g}")
                    nc.sync.dma_start(
                        t[:].rearrange("p (n u) -> p n u", n=4),
                        xT_sh.ap()
                        .rearrange("(n p) u -> n p u", p=P)[4 * g:4 * (g + 1)]
                        .rearrange("n p u -> p n u"),
                    )
                    xt.append(t)

                def xtile(c):
                    return xt[c // 4][:, (c % 4) * TS:(c % 4 + 1) * TS]

                def down_proj(wparam, group, nsub, bounce, row0):
                    w = p1w.tile([P, CCH * nsub * P], BF, name="p1w_t", tag="p1w_t")
                    nc.sync.dma_start(
                        w[:].rearrange("p (n m) -> p n m", n=CCH),
                        wparam[group].rearrange("(n p) m -> p n m", p=P),
                    )
                    for ls in range(nsub):
                        ps = p1ps.tile([P, TS], F32, name="p1ps_t", tag="p1ps_t")
                        for c in range(CCH):
                            nc.tensor.matmul(
                                ps[:],
                                w[:, c * nsub * P + ls * P:
                                  c * nsub * P + (ls + 1) * P],
                                xtile(c),
                                start=(c == 0),
                                stop=(c == CCH - 1),
                            )
                        sh = p1sh.tile([P, TS], BF, name="p1sh_t", tag="p1sh_t")
                        nc.scalar.copy(sh[:], ps[:])
                        nc.scalar.dma_start(
                            bounce[row0 + ls * P: row0 + (ls + 1) * P, :], sh[:]
                        )

                # c_kv (4 l-chunks) then k_r, then AG-kv
                down_proj(wdkvT, 0, 4, cc_in_kv, 0)
                wkr_sb = p1w.tile([P, CCH * DHR], BF, name="wkr_sb")
                nc.sync.dma_start(
                    wkr_sb[:].rearrange("p (n m) -> p n m", n=CCH),
                    wkrT.ap().rearrange("(n p) m -> p n m", p=P),
                )
                ps_kr = p1ps.tile([DHR, TS], F32, name="ps_kr", tag="p1ps_t")
                for c in range(CCH):
                    nc.tensor.matmul(
                        ps_kr[:],
                        wkr_sb[:, c * DHR:(c + 1) * DHR],
                        xtile(c),
                        start=(c == 0),
                        stop=(c == CCH - 1),
                    )
                sh_kr = p1sh.tile([DHR, TS], BF, name="sh_kr")
                nc.scalar.copy(sh_kr[:], ps_kr[:])
                nc.scalar.dma_start(cc_in_kv[NLKV:GKV, :], sh_kr[:])

                nc.gpsimd.collective_compute(
                    "AllGather",
                    mybir.AluOpType.bypass,
                    replica_groups=[list(range(NCORES))],
                    ins=[cc_in_kv.ap().opt()],
                    outs=[cc_out_kv.ap().opt()],
                )

                # c_q l-chunks 0-7 -> AG-qA, then 8-11 -> AG-qB, so the
                # q-proj accumulation chains start during the second mesh
                for g in range(2):
                    down_proj(wdqT, g, 4, cc_in_qa, g * 4 * P)
                nc.gpsimd.collective_compute(
                    "AllGather",
                    mybir.AluOpType.bypass,
                    replica_groups=[list(range(NCORES))],
                    ins=[cc_in_qa.ap().opt()],
                    outs=[cc_out_qa.ap().opt()],
                )
                down_proj(wdqT, 2, 4, cc_in_qb, 0)
                nc.gpsimd.collective_compute(
                    "AllGather",
                    mybir.AluOpType.bypass,
                    replica_groups=[list(range(NCORES))],
                    ins=[cc_in_qb.ap().opt()],
                    outs=[cc_out_qb.ap().opt()],
                )

            # ---- B = (W_uv.T @ W_o.T)[:, 2-head cols]  (independent of AGs) ----
            b_all = proj.tile([P, LKV * HPC * HS], BF)  # [128, 4*256]
            with (
                tc.tile_pool(name="pbw", bufs=3) as pbw,
                tc.tile_pool(name="pbps", bufs=1, space="PSUM") as pbps,
            ):
                ps_b = [
                    pbps.tile([P, HPC * HS], F32, name=f"ps_b{m}") for m in range(LKV)
                ]
                for c in range(CCH):
                    wuv_t = pbw.tile([P, NLKV], BF, name="wuv_t", tag="wuv_t")
                    nc.sync.dma_start(wuv_t[:], wuv[c])
                    wo_t = pbw.tile([P, HPC * HS], BF, name="wo_t", tag="wo_t")
                    nc.sync.dma_start(wo_t[:], woT[c])
                    for m in range(LKV):
                        nc.tensor.matmul(
                            ps_b[m][:],
                            wuv_t[:, m * P:(m + 1) * P],
                            wo_t[:],
                            start=(c == 0),
                            stop=(c == CCH - 1),
                        )
                for m in range(LKV):
                    nc.vector.tensor_copy(
                        b_all[:, m * HPC * HS:(m + 1) * HPC * HS], ps_b[m][:]
                    )

            # ---- prefetch post-gather projection weights (sync queue, before
            # the collective-gated latent loads) ----
            wuq_all = wts.tile([P, LQ * HPC * HS], BF)
            for l in range(LQ):
                nc.sync.dma_start(
                    wuq_all[:, l * HPC * HS:(l + 1) * HPC * HS], wuq[l]
                )
            wqr_all = wts.tile([P, LQ * HPC * DHR], BF)
            for l in range(LQ):
                nc.sync.dma_start(
                    wqr_all[:, l * HPC * DHR:(l + 1) * HPC * DHR], wqrT[l]
                )
            wuk_all = wts.tile([P, LKV * HPC * HS], BF)
            for l in range(LKV):
                nc.sync.dma_start(
                    wuk_all[:, l * HPC * HS:(l + 1) * HPC * HS], wukT[l]
                )

            with tc.tile_pool(name="rtmp", bufs=2) as rtmp:

                def rope(dst, src):
                    # dst = src * [cos;cos] + swap_halves(src) * [-sin;sin]
                    sw = rtmp.tile([DHR, T], BF, name="rsw", tag="rsw")
                    nc.sync.dma_start(sw[0:32, :], src[32:64, :])
                    nc.sync.dma_start(sw[32:64, :], src[0:32, :])
                    ta = rtmp.tile([DHR, T], BF, name="rta", tag="rta")
                    tb = rtmp.tile([DHR, T], BF, name="rtb", tag="rtb")
                    nc.vector.tensor_mul(ta[:], src, cos_sb[:])
                    nc.vector.tensor_mul(tb[:], sw[:], sin_sb[:])
                    nc.vector.tensor_add(dst, ta[:], tb[:])

                qT = proj.tile([P, HPC * T], BF)
                kT = proj.tile([P, HPC * T], BF)
                qr_rope = proj.tile([DHR, HPC * T], BF)
                qr_raw = proj.tile([DHR, HPC * T], BF)
                qr2 = proj.tile([P, T], BF)          # merged 2-head qr, pre-split
                v_sb = proj.tile([P, SC * HPC * HS], BF)
                kr_rope = proj.tile([DHR, T], BF)

                with tc.tile_pool(name="p5ps", bufs=5, space="PSUM") as p5ps:
                    # gathered kv latents (arrive second)
                    ckv_t = []
                    for l in range(LKV):
                        t = lat.tile([P, T], BF, name=f"ckv{l}", tag=f"ckv{l}")
                        nc.sync.dma_start(
                            t[:].rearrange("p (g u) -> p g u", g=NCORES),
                            cc_out_kv[:, l * P:(l + 1) * P, :].rearrange(
                                "g p u -> p g u"
                            ),
                        )
                        ckv_t.append(t)
                    kr_raw = lat.tile([DHR, T], BF)
                    nc.sync.dma_start(
                        kr_raw[:].rearrange("p (g u) -> p g u", g=NCORES),
                        cc_out_kv[:, NLKV:GKV, :].rearrange("g p u -> p g u"),
                    )
                    rope(kr_rope[:, :], kr_raw[:, :])

                    # k^T per head
                    for h in range(HPC):
                        for sj in range(TJ):
                            ps = p5ps.tile([P, 512], F32, name="ps_k", tag="p5")
                            for l in range(LKV):
                                nc.tensor.matmul(
                                    ps[:],
                                    wuk_all[:, l * HPC * HS + h * HS:
                                            l * HPC * HS + (h + 1) * HS],
                                    ckv_t[l][:, sj * 512:(sj + 1) * 512],
                                    start=(l == 0),
                                    stop=(l == LKV - 1),
                                )
                            nc.vector.tensor_copy(
                                kT[:, h * T + sj * 512: h * T + (sj + 1) * 512],
                                ps[:],
                            )
                    # v~ per s-chunk
                    for sc in range(SC):
                        ps = p5ps.tile([P, HPC * HS], F32, name="ps_v", tag="p5")
                        for l in range(LKV):
                            nc.tensor.matmul(
                                ps[:],
                                ckv_t[l][:, sc * P:(sc + 1) * P],
                                b_all[:, l * HPC * HS:(l + 1) * HPC * HS],
                                start=(l == 0),
                                stop=(l == LKV - 1),
                            )
                        nc.vector.tensor_copy(
                            v_sb[:, sc * HPC * HS:(sc + 1) * HPC * HS], ps[:]
                        )
                    # ---- gathered q latent (A half lands first) ----
                    cq_t = []
                    for l in range(LQ):
                        t = lat.tile([P, T], BF, name=f"cq{l}", tag=f"cq{l}")
                        if l < 8:
                            srcap = cc_out_qa[:, l * P:(l + 1) * P, :]
                        else:
                            srcap = cc_out_qb[:, (l - 8) * P:(l - 7) * P, :]
                        nc.sync.dma_start(
                            t[:].rearrange("p (g u) -> p g u", g=NCORES),
                            srcap.rearrange("g p u -> p g u"),
                        )
                        cq_t.append(t)

                    # q_r^T both heads in one matmul (M=128), split after
                    for tj in range(TJ):
                        ps = p5ps.tile([P, 512], F32, name="ps_qr", tag="p5")
                        for l in range(LQ):
                            nc.tensor.matmul(
                                ps[:],
                                wqr_all[:, l * HPC * DHR:(l + 1) * HPC * DHR],
                                cq_t[l][:, tj * 512:(tj + 1) * 512],
                                start=(l == 0),
                                stop=(l == LQ - 1),
                            )
                        nc.vector.tensor_copy(qr2[:, tj * 512:(tj + 1) * 512], ps[:])
                    nc.vector.tensor_copy(qr_raw[:, 0:T], qr2[0:DHR, :])
                    nc.sync.dma_start(qr_raw[:, T:HPC * T], qr2[DHR:P, :])
                    for h in range(HPC):
                        rope(qr_rope[:, h * T:(h + 1) * T],
                             qr_raw[:, h * T:(h + 1) * T])

                    # q^T per head
                    for h in range(HPC):
                        for tj in range(TJ):
                            ps = p5ps.tile([P, 512], F32, name="ps_q", tag="p5")
                            for l in range(LQ):
                                nc.tensor.matmul(
                                    ps[:],
                                    wuq_all[:, l * HPC * HS + h * HS:
                                            l * HPC * HS + (h + 1) * HS],
                                    cq_t[l][:, tj * 512:(tj + 1) * 512],
                                    start=(l == 0),
                                    stop=(l == LQ - 1),
                                )
                            nc.vector.tensor_copy(
                                qT[:, h * T + tj * 512: h * T + (tj + 1) * 512],
                                ps[:],
                            )


                # ---- attention (causal, per head, transposed-scores flow).
                # Denominator: DVE-accumulate exp tiles, one ones-matmul per
                # (h, tj) block instead of one per s-chunk.
                with (
                    tc.tile_pool(name="pss", bufs=5, space="PSUM") as pss,
                    tc.tile_pool(name="psy", bufs=2, space="PSUM") as psy,
                    tc.tile_pool(name="psx", bufs=1, space="PSUM") as psx,
                    tc.tile_pool(name="atp", bufs=8) as atp,
                    tc.tile_pool(name="accp", bufs=3) as accp,
                    tc.tile_pool(name="spool", bufs=3) as spool,
                    tc.tile_pool(name="opool", bufs=3) as opool,
                ):
                    for h in range(HPC):
                        for tj in range(TJ):
                            nsc = 4 * (tj + 1)
                            ps_y = psy.tile([P, 512], F32, name="ps_y", tag="psy")
                            acc = accp.tile([P, 512], F32, name="acc", tag="acc")
                            for k in range(nsc):
                                ps_s = pss.tile([P, 512], F32, name="ps_s", tag="pss")
                                nc.tensor.matmul(
                                    ps_s[:],
                                    kT[:, h * T + k * P: h * T + (k + 1) * P],
                                    qT[:, h * T + tj * 512: h * T + (tj + 1) * 512],
                                    start=True,
                                    stop=False,
                                )
                                nc.tensor.matmul(
                                    ps_s[:],
                                    kr_rope[:, k * P:(k + 1) * P],
                                    qr_rope[:, h * T + tj * 512:
                                            h * T + (tj + 1) * 512],
                                    start=False,
                                    stop=True,
                                )
                                m = k - 4 * tj
                                if m >= 0:
                                    nc.vector.tensor_add(
                                        ps_s[:], ps_s[:],
                                        cmask[:, m * 512:(m + 1) * 512],
                                    )
                                at = atp.tile([P, 512], BF, name="at", tag="at")
                                nc.scalar.activation(at[:], ps_s[:], Exp, scale=SCALE)
                                nc.tensor.matmul(
                                    ps_y[:],
                                    v_sb[:, k * HPC * HS + h * HS:
                                         k * HPC * HS + (h + 1) * HS],
                                    at[:],
                                    start=(k == 0),
                                    stop=(k == nsc - 1),
                                )
                                if k == 0:
                                    nc.vector.tensor_copy(acc[:], at[:])
                                else:
                                    nc.vector.tensor_add(acc[:], acc[:], at[:])
                            accb = spool.tile([P, 512], BF, name="accb", tag="accb")
                            nc.vector.tensor_copy(accb[:], acc[:])
                            ps_d = psx.tile([1, 512], F32, name="ps_d", tag="psx")
                            nc.tensor.matmul(ps_d[:], ones_bf[:], accb[:])
                            den_sb = spool.tile([1, 512], F32, name="den", tag="den")
                            nc.scalar.copy(den_sb[:], ps_d[:])
                            yT_sb = spool.tile([P, 512], BF, name="yT", tag="yT")
                            nc.scalar.copy(yT_sb[:], ps_y[:])
                            for u in range(4):
                                t0 = tj * 512 + u * P
                                ps_dt = psx.tile([P, 1], F32, name="ps_dt",
                                                 tag="psx")
                                nc.tensor.transpose(
                                    ps_dt[:], den_sb[:, u * P:(u + 1) * P],
                                    id_f32[:1, :1],
                                )
                                rec = spool.tile([P, 1], F32, name="rec", tag="rec")
                                nc.vector.reciprocal(rec[:], ps_dt[:])
                                ps_yt = psx.tile([P, P], BF, name="ps_yt",
                                                 tag="psx")
                                nc.tensor.transpose(
                                    ps_yt[:], yT_sb[:, u * P:(u + 1) * P], id_bf[:]
                                )
                                o_sb = opool.tile([P, HS], F32, name="o_sb", tag="o")
                                nc.scalar.activation(
                                    o_sb[:], ps_yt[:], Copy, scale=rec[:]
                                )
                                nc.sync.dma_start(
                                    out[h * T + t0: h * T + t0 + P, :], o_sb[:]
                                )
    nc.finalize()
    return nc


_ROPE_PERM = np.concatenate([np.arange(0, DHR, 2), np.arange(1, DHR, 2)])


def _bf(a):
    return np.ascontiguousarray(a).astype(ml_dtypes.bfloat16)


def _prep_inputs(x, freqs_cos, freqs_sin, W_dq, W_uq, W_dkv, W_uk, W_uv, W_qr,
                 W_kr, W_o):
    """Build the 8 per-core input maps (host-side layout prep, all bf16)."""
    x2 = np.asarray(x, np.float32).reshape(T, C)
    xT = x2.T                                        # [C, T]
    wdqT = _bf(np.asarray(W_dq).T.reshape(C, LQ // 4, 512).transpose(1, 0, 2))
    wdkvT = _bf(np.asarray(W_dkv).T.reshape(C, 1, 512).transpose(1, 0, 2))
    wkrT = _bf(np.asarray(W_kr)[_ROPE_PERM, :].T)    # [C, DHR], rope-permuted
    cosT = np.asarray(freqs_cos, np.float32).T       # [32, T]
    sinT = np.asarray(freqs_sin, np.float32).T
    cos2T = _bf(np.concatenate([cosT, cosT], axis=0))    # [64, T]
    sin2T = _bf(np.concatenate([-sinT, sinT], axis=0))
    wuq_full = np.asarray(W_uq).reshape(NLQ, NH * HS)
    wuv = _bf(np.asarray(W_uv).reshape(CCH, P, NLKV))
    W_qr_a = np.asarray(W_qr)
    W_uk_a = np.asarray(W_uk)
    W_o_a = np.asarray(W_o)

    in_maps = []
    for i in range(NCORES):
        h0 = i * HPC
        cols = slice(h0 * HS, (h0 + HPC) * HS)       # 256 output cols
        wqr_rows = np.concatenate(
            [W_qr_a[(h0 + h) * DHR + _ROPE_PERM, :] for h in range(HPC)], axis=0
        )                                            # [HPC*64=128, NLQ]
        in_maps.append({
            "xT_sh": _bf(xT[:, i * TS:(i + 1) * TS]),
            "wdqT": wdqT,
            "wdkvT": wdkvT,
            "wkrT": wkrT,
            "cos2T": cos2T,
            "sin2T": sin2T,
            "wuq": _bf(np.ascontiguousarray(wuq_full[:, cols])
                       .reshape(LQ, P, HPC * HS)),
            "wqrT": _bf(np.ascontiguousarray(wqr_rows.T)
                        .reshape(LQ, P, HPC * DHR)),
            "wukT": _bf(np.ascontiguousarray(
                        W_uk_a[h0 * HS:(h0 + HPC) * HS, :].T)
                        .reshape(LKV, P, HPC * HS)),
            "wuv": wuv,
            "woT": _bf(np.ascontiguousarray(W_o_a[cols, :].T)
                       .reshape(CCH, P, HPC * HS)),
        })
    return in_maps


_NC_CACHE = None


def kernel(**inputs):
    global _NC_CACHE
    in_maps = _prep_inputs(**inputs)
    if _NC_CACHE is None:
        _NC_CACHE = build_nc()
    res = run_bass_kernel_spmd(_NC_CACHE, in_maps, core_ids=list(range(NCORES)))
    outs = [np.asarray(res.results[i]["out"], np.float32)
            .reshape(HPC, T, HS).transpose(1, 0, 2).reshape(T, HPC * HS)
            for i in range(NCORES)]
    y = np.concatenate(outs, axis=1).reshape(B, T, C)
    return y



# revision 10
# speedup vs baseline: 1.3991x; 1.3991x over previous
"""MLA-style attention (nn_Attention_7868380086611) on 8 TRN2 NeuronCores.

Strategy (v2)
-------------
Head-parallel attention (2 of 16 heads per core).  The query path is fully
absorbed on the host into per-head combined weights (W_dq.T @ W_uq and
W_dq.T @ W_qr.T — weight-only products, same trick as the reference's own
v_eff absorption), so each core computes q/q_r for its 2 heads directly
from the full x with NO collective.  Only the tiny shared kv latent
(c_kv: 512 rows, k_r: 64 rows per token) is computed T-sharded and
AllGathered once (~288 KB per rank); the gather is overlapped with the
q-projection matmuls.  v_eff = W_uv.T @ W_o.T is also host-precomputed.

Per-core flow: load x^T (8.4 MB bf16) -> c_kv/k_r for its T/8 slice ->
AllGather trigger -> q^T/q_r^T for its 2 heads over full T (overlaps the
collective) -> k^T and v~ from the gathered latents -> causal attention
in transposed-score layout with PSUM-accumulated softmax denominator.
All inputs pre-cast/pre-tiled to bf16 on the host; PSUM accumulation fp32.
"""

import math
import sys

import numpy as np

sys.path.insert(0, "/opt/trn_rl_repo")

import ml_dtypes  # noqa: E402

from concourse import bacc, bass, masks, mybir  # noqa: E402
from concourse.bass_utils import run_bass_kernel_spmd  # noqa: E402
from concourse.tile import TileContext  # noqa: E402

B, T, C = 1, 2048, 2048
NH, HS = 16, 128
NLQ, NLKV, DHR = 1536, 512, 64
NCORES = 8
HPC = NH // NCORES          # heads per core = 2
TS = T // NCORES            # 256-token shard for the kv down-projection
P = 128
LKV = NLKV // P             # 4
CCH = C // P                # 16 c-chunks
TJ = T // 512               # 4 t-chunks of 512
SC = T // P                 # 16 s-chunks
SCALE = 1.0 / math.sqrt(HS + DHR)
NEG = -1.0e10

BF = mybir.dt.bfloat16
F32 = mybir.dt.float32
Exp = mybir.ActivationFunctionType.Exp
Copy = mybir.ActivationFunctionType.Copy

GKV = NLKV + DHR            # 576 rows in the all-gather buffer


def build_nc():
    nc = bacc.Bacc(None, target_bir_lowering=False, num_devices=NCORES)

    xTp = nc.declare_dram_parameter("xTp", [CCH, P, T], BF, isOutput=False)
    wdkvT = nc.declare_dram_parameter("wdkvT", [1, C, 512], BF, isOutput=False)
    wkrT = nc.declare_dram_parameter("wkrT", [C, DHR], BF, isOutput=False)
    cos2T = nc.declare_dram_parameter("cos2T", [DHR, T], BF, isOutput=False)
    sin2T = nc.declare_dram_parameter("sin2T", [DHR, T], BF, isOutput=False)
    wq = nc.declare_dram_parameter("wq", [CCH, P, HPC * HS], BF, isOutput=False)
    wqr = nc.declare_dram_parameter("wqr", [CCH, P, HPC * DHR], BF, isOutput=False)
    wukT = nc.declare_dram_parameter("wukT", [LKV, P, HPC * HS], BF, isOutput=False)
    bc = nc.declare_dram_parameter("bc", [LKV, P, HPC * HS], BF, isOutput=False)
    xs = nc.declare_dram_parameter("xs", [C, TS], BF, isOutput=False)
    out = nc.declare_dram_parameter("out", [HPC * T, HS], F32, isOutput=True)

    cc_in_kv = nc.dram_tensor("cc_in_kv", [GKV, TS], BF)
    cc_out_kv = nc.dram_tensor("cc_out_kv", [NCORES, GKV, TS], BF,
                               addr_space="Shared")

    with TileContext(nc) as tc:
        with (
            tc.tile_pool(name="persist", bufs=1) as persist,
            tc.tile_pool(name="lat", bufs=1) as lat,
            tc.tile_pool(name="proj", bufs=1) as proj,
            tc.tile_pool(name="wts", bufs=1) as wts,
        ):
            # ---- constants ----
            id_bf = persist.tile([P, P], BF)
            masks.make_identity(nc, id_bf[:])
            id_f32 = persist.tile([P, P], F32)
            masks.make_identity(nc, id_f32[:])
            ones_bf = persist.tile([P, 1], BF)
            nc.vector.memset(ones_bf[:], 1.0)
            # 4 additive causal masks [128, 512]: keep (0) iff t - s - 128*m >= 0
            cmask = persist.tile([P, 4 * 512], BF)
            nc.gpsimd.memset(cmask[:], 0.0)
            for m in range(4):
                nc.gpsimd.affine_select(
                    out=cmask[:, m * 512:(m + 1) * 512],
                    in_=cmask[:, m * 512:(m + 1) * 512],
                    compare_op=mybir.AluOpType.is_ge,
                    fill=NEG,
                    base=-m * P,
                    channel_multiplier=-1,
                    pattern=[[1, 512]],
                )
            cos_sb = persist.tile([DHR, T], BF)
            nc.scalar.dma_start(cos_sb[:], cos2T[:, :])
            sin_sb = persist.tile([DHR, T], BF)
            nc.scalar.dma_start(sin_sb[:], sin2T[:, :])

            # ---- phase 1: c_kv^T/k_r^T for own T/8 slice -> AllGather.
            # The rank-dependent x column slice comes in as a separate
            # pre-sliced input (xs) so the SPMD graph stays rank-independent.
            with (
                tc.tile_pool(name="p1wb", bufs=1) as p1w,
                tc.tile_pool(name="p1psb", bufs=3, space="PSUM") as p1ps,
                tc.tile_pool(name="p1shb", bufs=3) as p1sh,
                tc.tile_pool(name="p1xs", bufs=1) as p1xs,
            ):
                xsl = []
                for cgrp in range(4):
                    tsl = p1xs.tile([P, 4 * TS], BF, name=f"xsl{cgrp}",
                                    tag=f"xsl{cgrp}")
                    nc.sync.dma_start(
                        tsl[:].rearrange("p (n u) -> p n u", n=4),
                        xs.ap().rearrange("(n p) u -> n p u", p=P)
                        [4 * cgrp:4 * (cgrp + 1)].rearrange("n p u -> p n u"),
                    )
                    xsl.append(tsl)

                def xstile(c):
                    return xsl[c // 4][:, (c % 4) * TS:(c % 4 + 1) * TS]

                w = p1w.tile([P, CCH * 4 * P], BF, name="wdkv_sb")
                nc.sync.dma_start(
                    w[:].rearrange("p (n m) -> p n m", n=CCH),
                    wdkvT[0].rearrange("(n p) m -> p n m", p=P),
                )
                wkr_sb = p1w.tile([P, CCH * DHR], BF, name="wkr_sb")
                nc.sync.dma_start(
                    wkr_sb[:].rearrange("p (n m) -> p n m", n=CCH),
                    wkrT.ap().rearrange("(n p) m -> p n m", p=P),
                )
                for ls in range(LKV):
                    ps = p1ps.tile([P, TS], F32, name="p1ps_t", tag="p1ps_t")
                    for c in range(CCH):
                        nc.tensor.matmul(
                            ps[:],
                            w[:, c * 4 * P + ls * P: c * 4 * P + (ls + 1) * P],
                            xstile(c),
                            start=(c == 0),
                            stop=(c == CCH - 1),
                        )
                    sh = p1sh.tile([P, TS], BF, name="p1sh_t", tag="p1sh_t")
                    nc.scalar.copy(sh[:], ps[:])
                    nc.scalar.dma_start(
                        cc_in_kv[ls * P:(ls + 1) * P, :], sh[:]
                    )
                ps_kr = p1ps.tile([DHR, TS], F32, name="ps_kr", tag="p1ps_t")
                for c in range(CCH):
                    nc.tensor.matmul(
                        ps_kr[:],
                        wkr_sb[:, c * DHR:(c + 1) * DHR],
                        xstile(c),
                        start=(c == 0),
                        stop=(c == CCH - 1),
                    )
                sh_kr = p1sh.tile([DHR, TS], BF, name="sh_kr")
                nc.scalar.copy(sh_kr[:], ps_kr[:])
                nc.scalar.dma_start(cc_in_kv[NLKV:GKV, :], sh_kr[:])

                nc.gpsimd.collective_compute(
                    "AllGather",
                    mybir.AluOpType.bypass,
                    replica_groups=[list(range(NCORES))],
                    ins=[cc_in_kv.ap().opt()],
                    outs=[cc_out_kv.ap().opt()],
                )

            # ---- full x^T in SBUF (4 groups of 4 [128, T] chunks) ----
            xt = []
            for cgrp in range(4):
                t = lat.tile([P, 4 * T], BF, name=f"xt{cgrp}", tag=f"xt{cgrp}")
                nc.sync.dma_start(
                    t[:].rearrange("p (n u) -> p n u", n=4),
                    xTp.ap()[4 * cgrp:4 * (cgrp + 1)].rearrange("n p u -> p n u"),
                )
                xt.append(t)

            def xtile(c):
                return xt[c // 4][:, (c % 4) * T:(c % 4 + 1) * T]

            # ---- projection weights (no collective dependence) ----
            wq_all = wts.tile([P, CCH * HPC * HS], BF)
            nc.sync.dma_start(
                wq_all[:].rearrange("p (n m) -> p n m", n=CCH),
                wq.ap().rearrange("n p m -> p n m"),
            )
            wqr_all = wts.tile([P, CCH * HPC * DHR], BF)
            nc.sync.dma_start(
                wqr_all[:].rearrange("p (n m) -> p n m", n=CCH),
                wqr.ap().rearrange("n p m -> p n m"),
            )
            wuk_all = wts.tile([P, LKV * HPC * HS], BF)
            nc.sync.dma_start(
                wuk_all[:].rearrange("p (n m) -> p n m", n=LKV),
                wukT.ap().rearrange("n p m -> p n m"),
            )
            b_all = wts.tile([P, LKV * HPC * HS], BF)
            nc.sync.dma_start(
                b_all[:].rearrange("p (n m) -> p n m", n=LKV),
                bc.ap().rearrange("n p m -> p n m"),
            )

            with tc.tile_pool(name="rtmp", bufs=1) as rtmp:

                def rope(dst, src):
                    # dst = src * [cos;cos] + swap_halves(src) * [-sin;sin]
                    sw = rtmp.tile([DHR, T], BF, name="rsw", tag="rsw")
                    nc.sync.dma_start(sw[0:32, :], src[32:64, :])
                    nc.sync.dma_start(sw[32:64, :], src[0:32, :])
                    ta = rtmp.tile([DHR, T], BF, name="rta", tag="rta")
                    tb = rtmp.tile([DHR, T], BF, name="rtb", tag="rtb")
                    nc.vector.tensor_mul(ta[:], src, cos_sb[:])
                    nc.vector.tensor_mul(tb[:], sw[:], sin_sb[:])
                    nc.vector.tensor_add(dst, ta[:], tb[:])

                qT = proj.tile([P, HPC * T], BF)
                kT = proj.tile([P, HPC * T], BF)
                qr_rope = proj.tile([DHR, HPC * T], BF)
                qr2 = proj.tile([P, T], BF)          # merged 2-head qr, pre-split
                qr_h1 = proj.tile([DHR, T], BF)      # head-1 rows moved to part 0-63
                v_sb = proj.tile([P, SC * HPC * HS], BF)
                kr_rope = proj.tile([DHR, T], BF)

                with tc.tile_pool(name="p5ps", bufs=5, space="PSUM") as p5ps:
                    # q_r^T both heads in one matmul (M=128), split after
                    for tj in range(TJ):
                        ps = p5ps.tile([P, 512], F32, name="ps_qr", tag="p5")
                        for c in range(CCH):
                            nc.tensor.matmul(
                                ps[:],
                                wqr_all[:, c * HPC * DHR:(c + 1) * HPC * DHR],
                                xtile(c)[:, tj * 512:(tj + 1) * 512],
                                start=(c == 0),
                                stop=(c == CCH - 1),
                            )
                        nc.vector.tensor_copy(qr2[:, tj * 512:(tj + 1) * 512], ps[:])
                    nc.sync.dma_start(qr_h1[:, :], qr2[DHR:P, :])
                    rope(qr_rope[:, 0:T], qr2[0:DHR, :])
                    rope(qr_rope[:, T:HPC * T], qr_h1[:, :])

                    # q^T per head
                    for h in range(HPC):
                        for tj in range(TJ):
                            ps = p5ps.tile([P, 512], F32, name="ps_q", tag="p5")
                            for c in range(CCH):
                                nc.tensor.matmul(
                                    ps[:],
                                    wq_all[:, c * HPC * HS + h * HS:
                                           c * HPC * HS + (h + 1) * HS],
                                    xtile(c)[:, tj * 512:(tj + 1) * 512],
                                    start=(c == 0),
                                    stop=(c == CCH - 1),
                                )
                            nc.vector.tensor_copy(
                                qT[:, h * T + tj * 512: h * T + (tj + 1) * 512],
                                ps[:],
                            )

                    # ---- gathered kv latents ----
                    ckv_t = []
                    for l in range(LKV):
                        t = lat.tile([P, T], BF, name=f"ckv{l}", tag=f"ckv{l}")
                        nc.sync.dma_start(
                            t[:].rearrange("p (g u) -> p g u", g=NCORES),
                            cc_out_kv[:, l * P:(l + 1) * P, :].rearrange(
                                "g p u -> p g u"
                            ),
                        )
                        ckv_t.append(t)
                    kr_raw = lat.tile([DHR, T], BF)
                    nc.sync.dma_start(
                        kr_raw[:].rearrange("p (g u) -> p g u", g=NCORES),
                        cc_out_kv[:, NLKV:GKV, :].rearrange("g p u -> p g u"),
                    )
                    rope(kr_rope[:, :], kr_raw[:, :])

                    # k^T per head
                    for h in range(HPC):
                        for sj in range(TJ):
                            ps = p5ps.tile([P, 512], F32, name="ps_k", tag="p5")
                            for l in range(LKV):
                                nc.tensor.matmul(
                                    ps[:],
                                    wuk_all[:, l * HPC * HS + h * HS:
                                            l * HPC * HS + (h + 1) * HS],
                                    ckv_t[l][:, sj * 512:(sj + 1) * 512],
                                    start=(l == 0),
                                    stop=(l == LKV - 1),
                                )
                            nc.vector.tensor_copy(
                                kT[:, h * T + sj * 512: h * T + (sj + 1) * 512],
                                ps[:],
                            )
                    # v~ per s-chunk
                    for sc in range(SC):
                        ps = p5ps.tile([P, HPC * HS], F32, name="ps_v", tag="p5")
                        for l in range(LKV):
                            nc.tensor.matmul(
                                ps[:],
                                ckv_t[l][:, sc * P:(sc + 1) * P],
                                b_all[:, l * HPC * HS:(l + 1) * HPC * HS],
                                start=(l == 0),
                                stop=(l == LKV - 1),
                            )
                        nc.vector.tensor_copy(
                            v_sb[:, sc * HPC * HS:(sc + 1) * HPC * HS], ps[:]
                        )

                # ---- attention (causal, per head, transposed-scores flow) ----
                with (
                    tc.tile_pool(name="pss", bufs=5, space="PSUM") as pss,
                    tc.tile_pool(name="psy", bufs=2, space="PSUM") as psy,
                    tc.tile_pool(name="psx", bufs=1, space="PSUM") as psx,
                    tc.tile_pool(name="atp", bufs=8) as atp,
                    tc.tile_pool(name="accp", bufs=3) as accp,
                    tc.tile_pool(name="spool", bufs=3) as spool,
                    tc.tile_pool(name="opool", bufs=3) as opool,
                ):
                    for h in range(HPC):
                        for tj in range(TJ):
                            nsc = 4 * (tj + 1)
                            ps_y = psy.tile([P, 512], F32, name="ps_y", tag="psy")
                            acc = accp.tile([P, 512], F32, name="acc", tag="acc")
                            for k in range(nsc):
                                ps_s = pss.tile([P, 512], F32, name="ps_s", tag="pss")
                                nc.tensor.matmul(
                                    ps_s[:],
                                    kT[:, h * T + k * P: h * T + (k + 1) * P],
                                    qT[:, h * T + tj * 512: h * T + (tj + 1) * 512],
                                    start=True,
                                    stop=False,
                                )
                                nc.tensor.matmul(
                                    ps_s[:],
                                    kr_rope[:, k * P:(k + 1) * P],
                                    qr_rope[:, h * T + tj * 512:
                                            h * T + (tj + 1) * 512],
                                    start=False,
                                    stop=True,
                                )
                                m = k - 4 * tj
                                if m >= 0:
                                    nc.vector.tensor_add(
                                        ps_s[:], ps_s[:],
                                        cmask[:, m * 512:(m + 1) * 512],
                                    )
                                at = atp.tile([P, 512], BF, name="at", tag="at")
                                nc.scalar.activation(at[:], ps_s[:], Exp, scale=SCALE)
                                nc.tensor.matmul(
                                    ps_y[:],
                                    v_sb[:, k * HPC * HS + h * HS:
                                         k * HPC * HS + (h + 1) * HS],
                                    at[:],
                                    start=(k == 0),
                                    stop=(k == nsc - 1),
                                )
                                if k == 0:
                                    nc.vector.tensor_copy(acc[:], at[:])
                                else:
                                    nc.vector.tensor_add(acc[:], acc[:], at[:])
                            accb = spool.tile([P, 512], BF, name="accb", tag="accb")
                            nc.vector.tensor_copy(accb[:], acc[:])
                            ps_d = psx.tile([1, 512], F32, name="ps_d", tag="psx")
                            nc.tensor.matmul(ps_d[:], ones_bf[:], accb[:])
                            den_sb = spool.tile([1, 512], F32, name="den", tag="den")
                            nc.scalar.copy(den_sb[:], ps_d[:])
                            yT_sb = spool.tile([P, 512], BF, name="yT", tag="yT")
                            nc.scalar.copy(yT_sb[:], ps_y[:])
                            for u in range(4):
                                t0 = tj * 512 + u * P
                                ps_dt = psx.tile([P, 1], F32, name="ps_dt",
                                                 tag="psx")
                                nc.tensor.transpose(
                                    ps_dt[:], den_sb[:, u * P:(u + 1) * P],
                                    id_f32[:1, :1],
                                )
                                rec = spool.tile([P, 1], F32, name="rec", tag="rec")
                                nc.vector.reciprocal(rec[:], ps_dt[:])
                                ps_yt = psx.tile([P, P], BF, name="ps_yt",
                                                 tag="psx")
                                nc.tensor.transpose(
                                    ps_yt[:], yT_sb[:, u * P:(u + 1) * P], id_bf[:]
                                )
                                o_sb = opool.tile([P, HS], F32, name="o_sb", tag="o")
                                nc.scalar.activation(
                                    o_sb[:], ps_yt[:], Copy, scale=rec[:]
                                )
                                nc.sync.dma_start(
                                    out[h * T + t0: h * T + t0 + P, :], o_sb[:]
                                )
    nc.finalize()
    return nc


_ROPE_PERM = np.concatenate([np.arange(0, DHR, 2), np.arange(1, DHR, 2)])


def _bf(a):
    return np.ascontiguousarray(a).astype(ml_dtypes.bfloat16)


def _prep_inputs(x, freqs_cos, freqs_sin, W_dq, W_uq, W_dkv, W_uk, W_uv, W_qr,
                 W_kr, W_o):
    """Build the 8 per-core input maps (host-side layout prep, all bf16)."""
    x2 = np.asarray(x, np.float32).reshape(T, C)
    xT = np.ascontiguousarray(x2.T)                  # [C, T]
    xT_bf = _bf(xT).reshape(CCH, P, T)
    wdkvT = _bf(np.asarray(W_dkv).T.reshape(C, 1, 512).transpose(1, 0, 2))
    wkrT = _bf(np.asarray(W_kr)[_ROPE_PERM, :].T)    # [C, DHR], rope-permuted
    cosT = np.asarray(freqs_cos, np.float32).T       # [32, T]
    sinT = np.asarray(freqs_sin, np.float32).T
    cos2T = _bf(np.concatenate([cosT, cosT], axis=0))    # [64, T]
    sin2T = _bf(np.concatenate([-sinT, sinT], axis=0))

    Wdq = np.asarray(W_dq, np.float32)               # [NLQ, C]
    Wuq_mat = np.asarray(W_uq, np.float32).reshape(NLQ, NH * HS)
    Wq_comb = Wdq.T @ Wuq_mat                        # [C, NH*HS]
    Wqr_comb = Wdq.T @ np.asarray(W_qr, np.float32).T    # [C, NH*DHR]
    v_eff = np.asarray(W_uv, np.float32).T @ np.asarray(W_o, np.float32).T
    W_uk_a = np.asarray(W_uk)

    in_maps = []
    for i in range(NCORES):
        h0 = i * HPC
        cols = slice(h0 * HS, (h0 + HPC) * HS)       # 256 output cols
        wqr_cols = np.concatenate(
            [Wqr_comb[:, (h0 + h) * DHR + _ROPE_PERM] for h in range(HPC)],
            axis=1,
        )                                            # [C, HPC*64=128]
        in_maps.append({
            "xTp": xT_bf,
            "xs": _bf(xT[:, i * TS:(i + 1) * TS]),
            "wdkvT": wdkvT,
            "wkrT": wkrT,
            "cos2T": cos2T,
            "sin2T": sin2T,
            "wq": _bf(Wq_comb[:, cols]).reshape(CCH, P, HPC * HS),
            "wqr": _bf(wqr_cols).reshape(CCH, P, HPC * DHR),
            "wukT": _bf(np.ascontiguousarray(
                        W_uk_a[h0 * HS:(h0 + HPC) * HS, :].T)
                        .reshape(LKV, P, HPC * HS)),
            "bc": _bf(v_eff[:, cols]).reshape(LKV, P, HPC * HS),
        })
    return in_maps


_NC_CACHE = None


def kernel(**inputs):
    global _NC_CACHE
    in_maps = _prep_inputs(**inputs)
    if _NC_CACHE is None:
        _NC_CACHE = build_nc()
    res = run_bass_kernel_spmd(_NC_CACHE, in_maps, core_ids=list(range(NCORES)))
    outs = [np.asarray(res.results[i]["out"], np.float32)
            .reshape(HPC, T, HS).transpose(1, 0, 2).reshape(T, HPC * HS)
            for i in range(NCORES)]
    y = np.concatenate(outs, axis=1).reshape(B, T, C)
    return y


# revision 14
# speedup vs baseline: 1.4683x; 1.0495x over previous
"""MLA-style attention (nn_Attention_7868380086611) on 8 TRN2 NeuronCores.

Strategy (v3)
-------------
Head-parallel attention (2 of 16 heads per core).  The query path is fully
absorbed on the host into per-head combined weights (W_dq.T @ W_uq and
W_dq.T @ W_qr.T — weight-only products, same trick as the reference's own
v_eff absorption), so each core computes q/q_r for its 2 heads directly
from the full x with NO collective.  Only the tiny shared kv latent
(c_kv: 512 rows, k_r: 64 rows per token) is computed T-sharded and
AllGathered once (~288 KB per rank); the gather is overlapped with the
q-projection matmuls.  v_eff = W_uv.T @ W_o.T is host-precomputed.

v3 kernel-side improvements over v2:
- PE warm-up matmuls at t=0 so the HAM clock gate opens (2.4 GHz) before
  the real work starts.
- Attention is k-outer with software-pipelined AV matmuls (one k-chunk
  behind the score matmuls) so the tensor queue never stalls on exp; the
  stationary operand (kT/kr/v slice) is reused across the tj blocks of
  one k-chunk, cutting LDWEIGHTS count ~2.5x.
- Causal mask is a multiplicative bf16 0/1 mask applied to exp() output
  (vector 2x mode) instead of a -1e10 f32 add into PSUM (1x mode).
- Softmax denominator accumulates in bf16 (vector 2x) and one
  ones-matmul per (head, tj) on the bf16 accumulator.
- Projection loops are tj-inner so the stationary weight tile is reused
  across 4 matmuls (4x fewer LDWEIGHTS).
"""

import math
import sys

import numpy as np

sys.path.insert(0, "/opt/trn_rl_repo")

import ml_dtypes  # noqa: E402

from concourse import bacc, bass, masks, mybir  # noqa: E402
from concourse.bass_utils import run_bass_kernel_spmd  # noqa: E402
from concourse.tile import TileContext  # noqa: E402

B, T, C = 1, 2048, 2048
NH, HS = 16, 128
NLQ, NLKV, DHR = 1536, 512, 64
NCORES = 8
HPC = NH // NCORES          # heads per core = 2
TS = T // NCORES            # 256-token shard for the kv down-projection
P = 128
LKV = NLKV // P             # 4
CCH = C // P                # 16 c-chunks
TJ = T // 512               # 4 t-chunks of 512
SC = T // P                 # 16 s-chunks
SCALE = 1.0 / math.sqrt(HS + DHR)

BF = mybir.dt.bfloat16
F32 = mybir.dt.float32
Exp = mybir.ActivationFunctionType.Exp
Copy = mybir.ActivationFunctionType.Copy

GKV = NLKV + DHR            # 576 rows in the all-gather buffer


def build_nc():
    nc = bacc.Bacc(None, target_bir_lowering=False, num_devices=NCORES)

    xTp = nc.declare_dram_parameter("xTp", [CCH, P, T], BF, isOutput=False)
    wdkvT = nc.declare_dram_parameter("wdkvT", [1, C, 512], BF, isOutput=False)
    wkrT = nc.declare_dram_parameter("wkrT", [C, DHR], BF, isOutput=False)
    cos2T = nc.declare_dram_parameter("cos2T", [DHR, T], BF, isOutput=False)
    sin2T = nc.declare_dram_parameter("sin2T", [DHR, T], BF, isOutput=False)
    wq = nc.declare_dram_parameter("wq", [CCH, P, HPC * HS], BF, isOutput=False)
    wqr = nc.declare_dram_parameter("wqr", [CCH, P, HPC * DHR], BF, isOutput=False)
    wukT = nc.declare_dram_parameter("wukT", [LKV, P, HPC * HS], BF, isOutput=False)
    bc = nc.declare_dram_parameter("bc", [LKV, P, HPC * HS], BF, isOutput=False)
    xs = nc.declare_dram_parameter("xs", [C, TS], BF, isOutput=False)
    out = nc.declare_dram_parameter("out", [HPC * T, HS], F32, isOutput=True)

    cc_in_kv = nc.dram_tensor("cc_in_kv", [GKV, TS], BF)
    cc_out_kv = nc.dram_tensor("cc_out_kv", [NCORES, GKV, TS], BF,
                               addr_space="Shared")

    with TileContext(nc) as tc:
        with (
            tc.tile_pool(name="persist", bufs=1) as persist,
            tc.tile_pool(name="lat", bufs=1) as lat,
            tc.tile_pool(name="proj", bufs=1) as proj,
            tc.tile_pool(name="wts", bufs=1) as wts,
        ):
            # ---- constants ----
            id_bf = persist.tile([P, P], BF)
            masks.make_identity(nc, id_bf[:])
            id_f32 = persist.tile([P, P], F32)
            masks.make_identity(nc, id_f32[:])
            ones_bf = persist.tile([P, 1], BF)
            nc.vector.memset(ones_bf[:], 1.0)
            # 4 multiplicative causal masks [128, 512]: 1 iff t - s - 128*m >= 0
            mask01 = persist.tile([P, 4 * 512], BF)
            nc.vector.memset(mask01[:], 1.0)
            for m in range(4):
                nc.gpsimd.affine_select(
                    out=mask01[:, m * 512:(m + 1) * 512],
                    in_=mask01[:, m * 512:(m + 1) * 512],
                    compare_op=mybir.AluOpType.is_ge,
                    fill=0.0,
                    base=-m * P,
                    channel_multiplier=-1,
                    pattern=[[1, 512]],
                )
            cos_sb = persist.tile([DHR, T], BF)
            nc.scalar.dma_start(cos_sb[:], cos2T[:, :])
            sin_sb = persist.tile([DHR, T], BF)
            nc.scalar.dma_start(sin_sb[:], sin2T[:, :])

            # ---- phase 1: c_kv^T/k_r^T for own T/8 slice -> AllGather.
            # The rank-dependent x column slice comes in pre-sliced (xs) so
            # the SPMD graph stays rank-independent.
            with (
                tc.tile_pool(name="p1w", bufs=1) as p1w,
                tc.tile_pool(name="p1ps", bufs=2, space="PSUM") as p1ps,
                tc.tile_pool(name="p1sh", bufs=3) as p1sh,
                tc.tile_pool(name="p1xs", bufs=1) as p1xs,
            ):
                # PE warm-up: ~40 throwaway matmuls (~4us) so the HAM clock
                # gate opens before the first real accumulation chain.
                ps_w = p1ps.tile([P, P], F32, name="ps_warm", tag="warm")
                for _ in range(40):
                    nc.tensor.matmul(ps_w[:], id_bf[:], id_bf[:],
                                     start=True, stop=True)

                xsl = []
                for cgrp in range(4):
                    tsl = p1xs.tile([P, 4 * TS], BF, name=f"xsl{cgrp}",
                                    tag=f"xsl{cgrp}")
                    nc.sync.dma_start(
                        tsl[:].rearrange("p (n u) -> p n u", n=4),
                        xs.ap().rearrange("(n p) u -> n p u", p=P)
                        [4 * cgrp:4 * (cgrp + 1)].rearrange("n p u -> p n u"),
                    )
                    xsl.append(tsl)

                def xstile(c):
                    return xsl[c // 4][:, (c % 4) * TS:(c % 4 + 1) * TS]

                # wdkv in 4 group tiles so the first chain starts early
                w4 = []
                for g in range(4):
                    wt = p1w.tile([P, 4 * 512], BF, name=f"wdkv{g}",
                                  tag=f"wdkv{g}")
                    nc.sync.dma_start(
                        wt[:].rearrange("p (n m) -> p n m", n=4),
                        wdkvT[0].rearrange("(n p) m -> p n m", p=P)
                        [:, 4 * g:4 * (g + 1), :].rearrange("p n m -> p n m"),
                    )
                    w4.append(wt)
                wkr_sb = p1w.tile([P, CCH * DHR], BF, name="wkr_sb")
                nc.sync.dma_start(
                    wkr_sb[:].rearrange("p (n m) -> p n m", n=CCH),
                    wkrT.ap().rearrange("(n p) m -> p n m", p=P),
                )

                def wdkv_sl(c, ls):
                    return w4[c // 4][:, (c % 4) * 512 + ls * P:
                                      (c % 4) * 512 + (ls + 1) * P]

                for ls in range(LKV):
                    ps = p1ps.tile([P, TS], F32, name="p1ps_t", tag="p1ps_t")
                    for c in range(CCH):
                        nc.tensor.matmul(
                            ps[:], wdkv_sl(c, ls), xstile(c),
                            start=(c == 0), stop=(c == CCH - 1),
                        )
                    sh = p1sh.tile([P, TS], BF, name="p1sh_t", tag="p1sh_t")
                    nc.scalar.copy(sh[:], ps[:])
                    nc.scalar.dma_start(
                        cc_in_kv[ls * P:(ls + 1) * P, :], sh[:]
                    )
                ps_kr = p1ps.tile([DHR, TS], F32, name="ps_kr", tag="p1ps_t")
                for c in range(CCH):
                    nc.tensor.matmul(
                        ps_kr[:],
                        wkr_sb[:, c * DHR:(c + 1) * DHR],
                        xstile(c),
                        start=(c == 0),
                        stop=(c == CCH - 1),
                    )
                sh_kr = p1sh.tile([DHR, TS], BF, name="sh_kr")
                nc.scalar.copy(sh_kr[:], ps_kr[:])
                nc.scalar.dma_start(cc_in_kv[NLKV:GKV, :], sh_kr[:])

                nc.gpsimd.collective_compute(
                    "AllGather",
                    mybir.AluOpType.bypass,
                    replica_groups=[list(range(NCORES))],
                    ins=[cc_in_kv.ap().opt()],
                    outs=[cc_out_kv.ap().opt()],
                )

            # ---- projection weights, then full x^T (sync-queue order) ----
            wq_all = wts.tile([P, CCH * HPC * HS], BF)
            nc.sync.dma_start(
                wq_all[:].rearrange("p (n m) -> p n m", n=CCH),
                wq.ap().rearrange("n p m -> p n m"),
            )
            wqr_all = wts.tile([P, CCH * HPC * DHR], BF)
            nc.sync.dma_start(
                wqr_all[:].rearrange("p (n m) -> p n m", n=CCH),
                wqr.ap().rearrange("n p m -> p n m"),
            )
            xt = []
            for cgrp in range(4):
                t = lat.tile([P, 4 * T], BF, name=f"xt{cgrp}", tag=f"xt{cgrp}")
                nc.sync.dma_start(
                    t[:].rearrange("p (n u) -> p n u", n=4),
                    xTp.ap()[4 * cgrp:4 * (cgrp + 1)].rearrange("n p u -> p n u"),
                )
                xt.append(t)

            def xtile(c):
                return xt[c // 4][:, (c % 4) * T:(c % 4 + 1) * T]

            wuk_all = wts.tile([P, LKV * HPC * HS], BF)
            nc.sync.dma_start(
                wuk_all[:].rearrange("p (n m) -> p n m", n=LKV),
                wukT.ap().rearrange("n p m -> p n m"),
            )
            b_all = wts.tile([P, LKV * HPC * HS], BF)
            nc.sync.dma_start(
                b_all[:].rearrange("p (n m) -> p n m", n=LKV),
                bc.ap().rearrange("n p m -> p n m"),
            )

            with tc.tile_pool(name="rtmp", bufs=1) as rtmp:

                def rope(dst, src):
                    # dst = src * [cos;cos] + swap_halves(src) * [-sin;sin]
                    sw = rtmp.tile([DHR, T], BF, name="rsw", tag="rsw")
                    nc.sync.dma_start(sw[0:32, :], src[32:64, :])
                    nc.sync.dma_start(sw[32:64, :], src[0:32, :])
                    ta = rtmp.tile([DHR, T], BF, name="rta", tag="rta")
                    tb = rtmp.tile([DHR, T], BF, name="rtb", tag="rtb")
                    nc.vector.tensor_mul(ta[:], src, cos_sb[:])
                    nc.vector.tensor_mul(tb[:], sw[:], sin_sb[:])
                    nc.vector.tensor_add(dst, ta[:], tb[:])

                qT = proj.tile([P, HPC * T], BF)
                kT = proj.tile([P, HPC * T], BF)
                qr_rope = proj.tile([DHR, HPC * T], BF)
                qr2 = proj.tile([P, T], BF)          # merged 2-head qr, pre-split
                qr_h1 = proj.tile([DHR, T], BF)      # head-1 rows on part 0-63
                v_sb = proj.tile([P, SC * HPC * HS], BF)
                kr_rope = proj.tile([DHR, T], BF)

                with tc.tile_pool(name="p5ps", bufs=5, space="PSUM") as p5ps:
                    # q_r^T both heads in one pass (M=128), tj-inner so the
                    # stationary wqr chunk is loaded once per c
                    ps_qr = [
                        p5ps.tile([P, 512], F32, name=f"ps_qr{tj}", tag="p5")
                        for tj in range(TJ)
                    ]
                    for c in range(CCH):
                        for tj in range(TJ):
                            nc.tensor.matmul(
                                ps_qr[tj][:],
                                wqr_all[:, c * HPC * DHR:(c + 1) * HPC * DHR],
                                xtile(c)[:, tj * 512:(tj + 1) * 512],
                                start=(c == 0),
                                stop=(c == CCH - 1),
                            )
                    for tj in range(TJ):
                        nc.vector.tensor_copy(
                            qr2[:, tj * 512:(tj + 1) * 512], ps_qr[tj][:]
                        )
                    nc.sync.dma_start(qr_h1[:, :], qr2[DHR:P, :])
                    rope(qr_rope[:, 0:T], qr2[0:DHR, :])
                    rope(qr_rope[:, T:HPC * T], qr_h1[:, :])

                    # q^T per head, tj-inner
                    for h in range(HPC):
                        ps_q = [
                            p5ps.tile([P, 512], F32, name=f"ps_q{h}_{tj}",
                                      tag="p5")
                            for tj in range(TJ)
                        ]
                        for c in range(CCH):
                            for tj in range(TJ):
                                nc.tensor.matmul(
                                    ps_q[tj][:],
                                    wq_all[:, c * HPC * HS + h * HS:
                                           c * HPC * HS + (h + 1) * HS],
                                    xtile(c)[:, tj * 512:(tj + 1) * 512],
                                    start=(c == 0),
                                    stop=(c == CCH - 1),
                                )
                        for tj in range(TJ):
                            nc.scalar.copy(
                                qT[:, h * T + tj * 512: h * T + (tj + 1) * 512],
                                ps_q[tj][:],
                            )

                    # ---- gathered kv latents ----
                    ckv_t = []
                    for l in range(LKV):
                        t = lat.tile([P, T], BF, name=f"ckv{l}", tag=f"ckv{l}")
                        nc.sync.dma_start(
                            t[:].rearrange("p (g u) -> p g u", g=NCORES),
                            cc_out_kv[:, l * P:(l + 1) * P, :].rearrange(
                                "g p u -> p g u"
                            ),
                        )
                        ckv_t.append(t)
                    kr_raw = lat.tile([DHR, T], BF)
                    nc.sync.dma_start(
                        kr_raw[:].rearrange("p (g u) -> p g u", g=NCORES),
                        cc_out_kv[:, NLKV:GKV, :].rearrange("g p u -> p g u"),
                    )
                    rope(kr_rope[:, :], kr_raw[:, :])

                    # k^T per head, sj-inner (stationary wuk chunk reused)
                    for h in range(HPC):
                        ps_k = [
                            p5ps.tile([P, 512], F32, name=f"ps_k{h}_{sj}",
                                      tag="p5")
                            for sj in range(TJ)
                        ]
                        for l in range(LKV):
                            for sj in range(TJ):
                                nc.tensor.matmul(
                                    ps_k[sj][:],
                                    wuk_all[:, l * HPC * HS + h * HS:
                                            l * HPC * HS + (h + 1) * HS],
                                    ckv_t[l][:, sj * 512:(sj + 1) * 512],
                                    start=(l == 0),
                                    stop=(l == LKV - 1),
                                )
                        for sj in range(TJ):
                            nc.scalar.copy(
                                kT[:, h * T + sj * 512: h * T + (sj + 1) * 512],
                                ps_k[sj][:],
                            )
                    # v~ per s-chunk
                    for sc in range(SC):
                        ps = p5ps.tile([P, HPC * HS], F32, name="ps_v",
                                       tag="p5v", bufs=3)
                        for l in range(LKV):
                            nc.tensor.matmul(
                                ps[:],
                                ckv_t[l][:, sc * P:(sc + 1) * P],
                                b_all[:, l * HPC * HS:(l + 1) * HPC * HS],
                                start=(l == 0),
                                stop=(l == LKV - 1),
                            )
                        nc.vector.tensor_copy(
                            v_sb[:, sc * HPC * HS:(sc + 1) * HPC * HS], ps[:]
                        )

                # ---- attention (causal, k-outer, AV pipelined one k behind).
                with (
                    tc.tile_pool(name="psy", bufs=4, space="PSUM") as psy,
                    tc.tile_pool(name="pss", bufs=4, space="PSUM") as pss,
                    tc.tile_pool(name="atp", bufs=9) as atp,
                    tc.tile_pool(name="accp", bufs=6) as accp,
                    tc.tile_pool(name="spool", bufs=3) as spool,
                    tc.tile_pool(name="opool", bufs=3) as opool,
                ):
                    def vslice(k, h):
                        return v_sb[:, k * HPC * HS + h * HS:
                                    k * HPC * HS + (h + 1) * HS]

                    def tail(h, tj, ps_y, acc):
                        yT_sb = spool.tile([P, 512], BF, name="yT", tag="yT")
                        nc.scalar.copy(yT_sb[:], ps_y[:])
                        ps_d = pss.tile([1, 512], F32, name="ps_d", tag="pss")
                        nc.tensor.matmul(ps_d[:], ones_bf[:], acc[:],
                                         start=True, stop=True)
                        den_sb = spool.tile([1, 512], F32, name="den",
                                            tag="den")
                        nc.scalar.copy(den_sb[:], ps_d[:])
                        for u in range(4):
                            t0 = tj * 512 + u * P
                            ps_dt = pss.tile([P, 1], F32, name="ps_dt",
                                             tag="pss")
                            nc.tensor.transpose(
                                ps_dt[:], den_sb[:, u * P:(u + 1) * P],
                                id_f32[:1, :1],
                            )
                            rec = spool.tile([P, 1], F32, name="rec",
                                             tag="rec")
                            nc.vector.reciprocal(rec[:], ps_dt[:])
                            ps_yt = pss.tile([P, P], BF, name="ps_yt",
                                             tag="pss")
                            nc.tensor.transpose(
                                ps_yt[:], yT_sb[:, u * P:(u + 1) * P],
                                id_bf[:],
                            )
                            o_sb = opool.tile([P, HS], F32, name="o_sb",
                                              tag="o")
                            nc.scalar.activation(
                                o_sb[:], ps_yt[:], Copy, scale=rec[:]
                            )
                            nc.sync.dma_start(
                                out[h * T + t0: h * T + t0 + P, :], o_sb[:]
                            )

                    for h in range(HPC):
                        ps_y = {
                            tj: psy.tile([P, 512], F32, name=f"psy{h}_{tj}",
                                         tag="psy")
                            for tj in range(TJ)
                        }
                        acc = {
                            tj: accp.tile([P, 512], BF, name=f"acc{h}_{tj}",
                                          tag="acc")
                            for tj in range(TJ)
                        }
                        pend = {}

                        def emit_av(k):
                            # AV matmuls for chunk k (stationary v reused)
                            for tj, at_prev in pend.pop(k).items():
                                nc.tensor.matmul(
                                    ps_y[tj][:], vslice(k, h), at_prev[:],
                                    start=(k == 0), stop=(k == 4 * tj + 3),
                                )
                            # drain any (h, tj) whose last AV just ran
                            if k >= 3 and (k - 3) % 4 == 0:
                                tjd = (k - 3) // 4
                                tail(h, tjd, ps_y[tjd], acc[tjd])

                        for k in range(SC):
                            tjs = list(range(k // 4, TJ))
                            ats = {}
                            # sub-groups of <=3 so the 3-deep pss ring can't
                            # deadlock; stationary reused within each group
                            for gi in range(0, len(tjs), 3):
                                grp = tjs[gi:gi + 3]
                                ps_t = {}
                                for tj in grp:
                                    ps_s = pss.tile([P, 512], F32,
                                                    name="ps_s", tag="pss")
                                    nc.tensor.matmul(
                                        ps_s[:],
                                        kT[:, h * T + k * P:
                                           h * T + (k + 1) * P],
                                        qT[:, h * T + tj * 512:
                                           h * T + (tj + 1) * 512],
                                        start=True, stop=False,
                                    )
                                    ps_t[tj] = ps_s
                                for tj in grp:
                                    nc.tensor.matmul(
                                        ps_t[tj][:],
                                        kr_rope[:, k * P:(k + 1) * P],
                                        qr_rope[:, h * T + tj * 512:
                                                h * T + (tj + 1) * 512],
                                        start=False, stop=True,
                                    )
                                for tj in grp:
                                    at = atp.tile([P, 512], BF, name="at",
                                                  tag="at")
                                    nc.scalar.activation(
                                        at[:], ps_t[tj][:], Exp, scale=SCALE
                                    )
                                    if tj == k // 4:
                                        nc.vector.tensor_mul(
                                            at[:], at[:],
                                            mask01[:, (k % 4) * 512:
                                                   (k % 4 + 1) * 512],
                                        )
                                    if k == 0:
                                        nc.vector.tensor_copy(acc[tj][:],
                                                              at[:])
                                    else:
                                        nc.vector.tensor_add(
                                            acc[tj][:], acc[tj][:], at[:]
                                        )
                                    ats[tj] = at
                            pend[k] = ats
                            if k - 1 in pend:
                                emit_av(k - 1)
                        emit_av(SC - 1)
    nc.finalize()
    return nc


_ROPE_PERM = np.concatenate([np.arange(0, DHR, 2), np.arange(1, DHR, 2)])


def _bf(a):
    return np.ascontiguousarray(a).astype(ml_dtypes.bfloat16)


def _prep_inputs(x, freqs_cos, freqs_sin, W_dq, W_uq, W_dkv, W_uk, W_uv, W_qr,
                 W_kr, W_o):
    """Build the 8 per-core input maps (host-side layout prep, all bf16)."""
    x2 = np.asarray(x, np.float32).reshape(T, C)
    xT = np.ascontiguousarray(x2.T)                  # [C, T]
    xT_bf = _bf(xT).reshape(CCH, P, T)
    wdkvT = _bf(np.asarray(W_dkv).T.reshape(C, 1, 512).transpose(1, 0, 2))
    wkrT = _bf(np.asarray(W_kr)[_ROPE_PERM, :].T)    # [C, DHR], rope-permuted
    cosT = np.asarray(freqs_cos, np.float32).T       # [32, T]
    sinT = np.asarray(freqs_sin, np.float32).T
    cos2T = _bf(np.concatenate([cosT, cosT], axis=0))    # [64, T]
    sin2T = _bf(np.concatenate([-sinT, sinT], axis=0))

    Wdq = np.asarray(W_dq, np.float32)               # [NLQ, C]
    Wuq_mat = np.asarray(W_uq, np.float32).reshape(NLQ, NH * HS)
    Wq_comb = Wdq.T @ Wuq_mat                        # [C, NH*HS]
    Wqr_comb = Wdq.T @ np.asarray(W_qr, np.float32).T    # [C, NH*DHR]
    v_eff = np.asarray(W_uv, np.float32).T @ np.asarray(W_o, np.float32).T
    W_uk_a = np.asarray(W_uk)

    in_maps = []
    for i in range(NCORES):
        h0 = i * HPC
        cols = slice(h0 * HS, (h0 + HPC) * HS)       # 256 output cols
        wqr_cols = np.concatenate(
            [Wqr_comb[:, (h0 + h) * DHR + _ROPE_PERM] for h in range(HPC)],
            axis=1,
        )                                            # [C, HPC*64=128]
        in_maps.append({
            "xTp": xT_bf,
            "xs": _bf(xT[:, i * TS:(i + 1) * TS]),
            "wdkvT": wdkvT,
            "wkrT": wkrT,
            "cos2T": cos2T,
            "sin2T": sin2T,
            "wq": _bf(Wq_comb[:, cols]).reshape(CCH, P, HPC * HS),
            "wqr": _bf(wqr_cols).reshape(CCH, P, HPC * DHR),
            "wukT": _bf(np.ascontiguousarray(
                        W_uk_a[h0 * HS:(h0 + HPC) * HS, :].T)
                        .reshape(LKV, P, HPC * HS)),
            "bc": _bf(v_eff[:, cols]).reshape(LKV, P, HPC * HS),
        })
    return in_maps


_NC_CACHE = None


def kernel(**inputs):
    global _NC_CACHE
    in_maps = _prep_inputs(**inputs)
    if _NC_CACHE is None:
        _NC_CACHE = build_nc()
    res = run_bass_kernel_spmd(_NC_CACHE, in_maps, core_ids=list(range(NCORES)))
    outs = [np.asarray(res.results[i]["out"], np.float32)
            .reshape(HPC, T, HS).transpose(1, 0, 2).reshape(T, HPC * HS)
            for i in range(NCORES)]
    y = np.concatenate(outs, axis=1).reshape(B, T, C)
    return y


# revision 16
# speedup vs baseline: 1.4731x; 1.0033x over previous
"""MLA-style attention (nn_Attention_7868380086611) on 8 TRN2 NeuronCores.

Strategy (v3)
-------------
Head-parallel attention (2 of 16 heads per core).  The query path is fully
absorbed on the host into per-head combined weights (W_dq.T @ W_uq and
W_dq.T @ W_qr.T — weight-only products, same trick as the reference's own
v_eff absorption), so each core computes q/q_r for its 2 heads directly
from the full x with NO collective.  Only the tiny shared kv latent
(c_kv: 512 rows, k_r: 64 rows per token) is computed T-sharded and
AllGathered once (~288 KB per rank); the gather is overlapped with the
q-projection matmuls.  v_eff = W_uv.T @ W_o.T is host-precomputed.

v3 kernel-side improvements over v2:
- PE warm-up matmuls at t=0 so the HAM clock gate opens (2.4 GHz) before
  the real work starts.
- Attention is k-outer with software-pipelined AV matmuls (one k-chunk
  behind the score matmuls) so the tensor queue never stalls on exp; the
  stationary operand (kT/kr/v slice) is reused across the tj blocks of
  one k-chunk, cutting LDWEIGHTS count ~2.5x.
- Causal mask is a multiplicative bf16 0/1 mask applied to exp() output
  (vector 2x mode) instead of a -1e10 f32 add into PSUM (1x mode).
- Softmax denominator accumulates in bf16 (vector 2x) and one
  ones-matmul per (head, tj) on the bf16 accumulator.
- Projection loops are tj-inner so the stationary weight tile is reused
  across 4 matmuls (4x fewer LDWEIGHTS).
"""

import math
import sys

import numpy as np

sys.path.insert(0, "/opt/trn_rl_repo")

import ml_dtypes  # noqa: E402

from concourse import bacc, bass, masks, mybir  # noqa: E402
from concourse.bass_utils import run_bass_kernel_spmd  # noqa: E402
from concourse.tile import TileContext  # noqa: E402

B, T, C = 1, 2048, 2048
NH, HS = 16, 128
NLQ, NLKV, DHR = 1536, 512, 64
NCORES = 8
HPC = NH // NCORES          # heads per core = 2
TS = T // NCORES            # 256-token shard for the kv down-projection
P = 128
LKV = NLKV // P             # 4
CCH = C // P                # 16 c-chunks
TJ = T // 512               # 4 t-chunks of 512
SC = T // P                 # 16 s-chunks
SCALE = 1.0 / math.sqrt(HS + DHR)

BF = mybir.dt.bfloat16
F32 = mybir.dt.float32
Exp = mybir.ActivationFunctionType.Exp
Copy = mybir.ActivationFunctionType.Copy

GKV = NLKV + DHR            # 576 rows in the all-gather buffer


def build_nc():
    nc = bacc.Bacc(None, target_bir_lowering=False, num_devices=NCORES)

    xTp = nc.declare_dram_parameter("xTp", [CCH, P, T], BF, isOutput=False)
    wdkvT = nc.declare_dram_parameter("wdkvT", [1, C, 512], BF, isOutput=False)
    wkrT = nc.declare_dram_parameter("wkrT", [C, DHR], BF, isOutput=False)
    cos2T = nc.declare_dram_parameter("cos2T", [DHR, T], BF, isOutput=False)
    sin2T = nc.declare_dram_parameter("sin2T", [DHR, T], BF, isOutput=False)
    wq = nc.declare_dram_parameter("wq", [CCH, P, HPC * HS], BF, isOutput=False)
    wqr = nc.declare_dram_parameter("wqr", [CCH, P, HPC * DHR], BF, isOutput=False)
    wukT = nc.declare_dram_parameter("wukT", [LKV, P, HPC * HS], BF, isOutput=False)
    bc = nc.declare_dram_parameter("bc", [LKV, P, HPC * HS], BF, isOutput=False)
    xs = nc.declare_dram_parameter("xs", [C, TS], BF, isOutput=False)
    out = nc.declare_dram_parameter("out", [HPC * T, HS], F32, isOutput=True)

    cc_in_kv = nc.dram_tensor("cc_in_kv", [GKV, TS], BF)
    cc_out_kv = nc.dram_tensor("cc_out_kv", [NCORES, GKV, TS], BF,
                               addr_space="Shared")

    with TileContext(nc) as tc:
        with (
            tc.tile_pool(name="persist", bufs=1) as persist,
            tc.tile_pool(name="lat", bufs=1) as lat,
            tc.tile_pool(name="proj", bufs=1) as proj,
            tc.tile_pool(name="wts", bufs=1) as wts,
        ):
            # ---- constants ----
            id_bf = persist.tile([P, P], BF)
            masks.make_identity(nc, id_bf[:])
            id_f32 = persist.tile([P, P], F32)
            masks.make_identity(nc, id_f32[:])
            ones_bf = persist.tile([P, 1], BF)
            nc.vector.memset(ones_bf[:], 1.0)
            # 4 multiplicative causal masks [128, 512]: 1 iff t - s - 128*m >= 0
            mask01 = persist.tile([P, 4 * 512], BF)
            nc.vector.memset(mask01[:], 1.0)
            for m in range(4):
                nc.gpsimd.affine_select(
                    out=mask01[:, m * 512:(m + 1) * 512],
                    in_=mask01[:, m * 512:(m + 1) * 512],
                    compare_op=mybir.AluOpType.is_ge,
                    fill=0.0,
                    base=-m * P,
                    channel_multiplier=-1,
                    pattern=[[1, 512]],
                )
            # cos/sin on the gpsimd queue: keeps the scalar queue free for
            # the phase-1 PSUM drains + bounce stores that gate the AllGather
            cos_sb = persist.tile([DHR, T], BF)
            nc.gpsimd.dma_start(cos_sb[:], cos2T[:, :])
            sin_sb = persist.tile([DHR, T], BF)
            nc.gpsimd.dma_start(sin_sb[:], sin2T[:, :])

            # ---- phase 1: c_kv^T/k_r^T for own T/8 slice -> AllGather.
            # The rank-dependent x column slice comes in pre-sliced (xs) so
            # the SPMD graph stays rank-independent.
            with (
                tc.tile_pool(name="p1w", bufs=1) as p1w,
                tc.tile_pool(name="p1ps", bufs=2, space="PSUM") as p1ps,
                tc.tile_pool(name="p1sh", bufs=3) as p1sh,
                tc.tile_pool(name="p1xs", bufs=1) as p1xs,
            ):
                # PE warm-up: N=512 throwaway matmuls bridging until the xs
                # DMA lands (~9us) so the HAM clock gate opens (2.4 GHz) and
                # STAYS open into the first real accumulation chain.
                junk = p1w.tile([P, 512], BF, name="junk")
                nc.vector.memset(junk[:], 0.0)
                ps_w = p1ps.tile([P, 512], F32, name="ps_warm", tag="warm")
                for _ in range(36):
                    nc.tensor.matmul(ps_w[:], id_bf[:], junk[:],
                                     start=True, stop=True)

                xsl = []
                for cgrp in range(4):
                    tsl = p1xs.tile([P, 4 * TS], BF, name=f"xsl{cgrp}",
                                    tag=f"xsl{cgrp}")
                    nc.sync.dma_start(
                        tsl[:].rearrange("p (n u) -> p n u", n=4),
                        xs.ap().rearrange("(n p) u -> n p u", p=P)
                        [4 * cgrp:4 * (cgrp + 1)].rearrange("n p u -> p n u"),
                    )
                    xsl.append(tsl)

                def xstile(c):
                    return xsl[c // 4][:, (c % 4) * TS:(c % 4 + 1) * TS]

                # wdkv in 4 group tiles so the first chain starts early
                w4 = []
                for g in range(4):
                    wt = p1w.tile([P, 4 * 512], BF, name=f"wdkv{g}",
                                  tag=f"wdkv{g}")
                    nc.sync.dma_start(
                        wt[:].rearrange("p (n m) -> p n m", n=4),
                        wdkvT[0].rearrange("(n p) m -> p n m", p=P)
                        [:, 4 * g:4 * (g + 1), :].rearrange("p n m -> p n m"),
                    )
                    w4.append(wt)
                wkr_sb = p1w.tile([P, CCH * DHR], BF, name="wkr_sb")
                nc.sync.dma_start(
                    wkr_sb[:].rearrange("p (n m) -> p n m", n=CCH),
                    wkrT.ap().rearrange("(n p) m -> p n m", p=P),
                )

                def wdkv_sl(c, ls):
                    return w4[c // 4][:, (c % 4) * 512 + ls * P:
                                      (c % 4) * 512 + (ls + 1) * P]

                for ls in range(LKV):
                    ps = p1ps.tile([P, TS], F32, name="p1ps_t", tag="p1ps_t")
                    for c in range(CCH):
                        nc.tensor.matmul(
                            ps[:], wdkv_sl(c, ls), xstile(c),
                            start=(c == 0), stop=(c == CCH - 1),
                        )
                    sh = p1sh.tile([P, TS], BF, name="p1sh_t", tag="p1sh_t")
                    nc.scalar.copy(sh[:], ps[:])
                    nc.scalar.dma_start(
                        cc_in_kv[ls * P:(ls + 1) * P, :], sh[:]
                    )
                ps_kr = p1ps.tile([DHR, TS], F32, name="ps_kr", tag="p1ps_t")
                for c in range(CCH):
                    nc.tensor.matmul(
                        ps_kr[:],
                        wkr_sb[:, c * DHR:(c + 1) * DHR],
                        xstile(c),
                        start=(c == 0),
                        stop=(c == CCH - 1),
                    )
                sh_kr = p1sh.tile([DHR, TS], BF, name="sh_kr")
                nc.scalar.copy(sh_kr[:], ps_kr[:])
                nc.scalar.dma_start(cc_in_kv[NLKV:GKV, :], sh_kr[:])

                nc.gpsimd.collective_compute(
                    "AllGather",
                    mybir.AluOpType.bypass,
                    replica_groups=[list(range(NCORES))],
                    ins=[cc_in_kv.ap().opt()],
                    outs=[cc_out_kv.ap().opt()],
                )

            # ---- projection weights, then full x^T (sync-queue order) ----
            wq_all = wts.tile([P, CCH * HPC * HS], BF)
            nc.sync.dma_start(
                wq_all[:].rearrange("p (n m) -> p n m", n=CCH),
                wq.ap().rearrange("n p m -> p n m"),
            )
            wqr_all = wts.tile([P, CCH * HPC * DHR], BF)
            nc.sync.dma_start(
                wqr_all[:].rearrange("p (n m) -> p n m", n=CCH),
                wqr.ap().rearrange("n p m -> p n m"),
            )
            xt = []
            for cgrp in range(4):
                t = lat.tile([P, 4 * T], BF, name=f"xt{cgrp}", tag=f"xt{cgrp}")
                nc.sync.dma_start(
                    t[:].rearrange("p (n u) -> p n u", n=4),
                    xTp.ap()[4 * cgrp:4 * (cgrp + 1)].rearrange("n p u -> p n u"),
                )
                xt.append(t)

            def xtile(c):
                return xt[c // 4][:, (c % 4) * T:(c % 4 + 1) * T]

            wuk_all = wts.tile([P, LKV * HPC * HS], BF)
            nc.sync.dma_start(
                wuk_all[:].rearrange("p (n m) -> p n m", n=LKV),
                wukT.ap().rearrange("n p m -> p n m"),
            )
            b_all = wts.tile([P, LKV * HPC * HS], BF)
            nc.sync.dma_start(
                b_all[:].rearrange("p (n m) -> p n m", n=LKV),
                bc.ap().rearrange("n p m -> p n m"),
            )

            with tc.tile_pool(name="rtmp", bufs=1) as rtmp:

                def rope(dst, src):
                    # dst = src * [cos;cos] + swap_halves(src) * [-sin;sin]
                    sw = rtmp.tile([DHR, T], BF, name="rsw", tag="rsw")
                    nc.sync.dma_start(sw[0:32, :], src[32:64, :])
                    nc.sync.dma_start(sw[32:64, :], src[0:32, :])
                    ta = rtmp.tile([DHR, T], BF, name="rta", tag="rta")
                    tb = rtmp.tile([DHR, T], BF, name="rtb", tag="rtb")
                    nc.vector.tensor_mul(ta[:], src, cos_sb[:])
                    nc.vector.tensor_mul(tb[:], sw[:], sin_sb[:])
                    nc.vector.tensor_add(dst, ta[:], tb[:])

                qT = proj.tile([P, HPC * T], BF)
                kT = proj.tile([P, HPC * T], BF)
                qr_rope = proj.tile([DHR, HPC * T], BF)
                qr2 = proj.tile([P, T], BF)          # merged 2-head qr, pre-split
                qr_h1 = proj.tile([DHR, T], BF)      # head-1 rows on part 0-63
                v_sb = proj.tile([P, SC * HPC * HS], BF)
                kr_rope = proj.tile([DHR, T], BF)

                with tc.tile_pool(name="p5ps", bufs=5, space="PSUM") as p5ps:
                    # q_r^T both heads in one pass (M=128), tj-inner so the
                    # stationary wqr chunk is loaded once per c
                    ps_qr = [
                        p5ps.tile([P, 512], F32, name=f"ps_qr{tj}", tag="p5")
                        for tj in range(TJ)
                    ]
                    for c in range(CCH):
                        for tj in range(TJ):
                            nc.tensor.matmul(
                                ps_qr[tj][:],
                                wqr_all[:, c * HPC * DHR:(c + 1) * HPC * DHR],
                                xtile(c)[:, tj * 512:(tj + 1) * 512],
                                start=(c == 0),
                                stop=(c == CCH - 1),
                            )
                    for tj in range(TJ):
                        nc.vector.tensor_copy(
                            qr2[:, tj * 512:(tj + 1) * 512], ps_qr[tj][:]
                        )
                    nc.sync.dma_start(qr_h1[:, :], qr2[DHR:P, :])
                    rope(qr_rope[:, 0:T], qr2[0:DHR, :])
                    rope(qr_rope[:, T:HPC * T], qr_h1[:, :])

                    # q^T per head, tj-inner
                    for h in range(HPC):
                        ps_q = [
                            p5ps.tile([P, 512], F32, name=f"ps_q{h}_{tj}",
                                      tag="p5")
                            for tj in range(TJ)
                        ]
                        for c in range(CCH):
                            for tj in range(TJ):
                                nc.tensor.matmul(
                                    ps_q[tj][:],
                                    wq_all[:, c * HPC * HS + h * HS:
                                           c * HPC * HS + (h + 1) * HS],
                                    xtile(c)[:, tj * 512:(tj + 1) * 512],
                                    start=(c == 0),
                                    stop=(c == CCH - 1),
                                )
                        for tj in range(TJ):
                            nc.scalar.copy(
                                qT[:, h * T + tj * 512: h * T + (tj + 1) * 512],
                                ps_q[tj][:],
                            )

                    # ---- gathered kv latents ----
                    ckv_t = []
                    for l in range(LKV):
                        t = lat.tile([P, T], BF, name=f"ckv{l}", tag=f"ckv{l}")
                        nc.sync.dma_start(
                            t[:].rearrange("p (g u) -> p g u", g=NCORES),
                            cc_out_kv[:, l * P:(l + 1) * P, :].rearrange(
                                "g p u -> p g u"
                            ),
                        )
                        ckv_t.append(t)
                    kr_raw = lat.tile([DHR, T], BF)
                    nc.sync.dma_start(
                        kr_raw[:].rearrange("p (g u) -> p g u", g=NCORES),
                        cc_out_kv[:, NLKV:GKV, :].rearrange("g p u -> p g u"),
                    )
                    rope(kr_rope[:, :], kr_raw[:, :])

                    # k^T per head, sj-inner (stationary wuk chunk reused)
                    for h in range(HPC):
                        ps_k = [
                            p5ps.tile([P, 512], F32, name=f"ps_k{h}_{sj}",
                                      tag="p5")
                            for sj in range(TJ)
                        ]
                        for l in range(LKV):
                            for sj in range(TJ):
                                nc.tensor.matmul(
                                    ps_k[sj][:],
                                    wuk_all[:, l * HPC * HS + h * HS:
                                            l * HPC * HS + (h + 1) * HS],
                                    ckv_t[l][:, sj * 512:(sj + 1) * 512],
                                    start=(l == 0),
                                    stop=(l == LKV - 1),
                                )
                        for sj in range(TJ):
                            nc.scalar.copy(
                                kT[:, h * T + sj * 512: h * T + (sj + 1) * 512],
                                ps_k[sj][:],
                            )
                    # v~ per s-chunk
                    for sc in range(SC):
                        ps = p5ps.tile([P, HPC * HS], F32, name="ps_v",
                                       tag="p5v", bufs=3)
                        for l in range(LKV):
                            nc.tensor.matmul(
                                ps[:],
                                ckv_t[l][:, sc * P:(sc + 1) * P],
                                b_all[:, l * HPC * HS:(l + 1) * HPC * HS],
                                start=(l == 0),
                                stop=(l == LKV - 1),
                            )
                        nc.vector.tensor_copy(
                            v_sb[:, sc * HPC * HS:(sc + 1) * HPC * HS], ps[:]
                        )

                # ---- attention (causal, k-outer, AV pipelined one k behind).
                with (
                    tc.tile_pool(name="psy", bufs=4, space="PSUM") as psy,
                    tc.tile_pool(name="pss", bufs=4, space="PSUM") as pss,
                    tc.tile_pool(name="atp", bufs=9) as atp,
                    tc.tile_pool(name="accp", bufs=6) as accp,
                    tc.tile_pool(name="spool", bufs=3) as spool,
                    tc.tile_pool(name="opool", bufs=3) as opool,
                ):
                    def vslice(k, h):
                        return v_sb[:, k * HPC * HS + h * HS:
                                    k * HPC * HS + (h + 1) * HS]

                    def tail(h, tj, ps_y, acc):
                        yT_sb = spool.tile([P, 512], BF, name="yT", tag="yT")
                        nc.scalar.copy(yT_sb[:], ps_y[:])
                        ps_d = pss.tile([1, 512], F32, name="ps_d", tag="pss")
                        nc.tensor.matmul(ps_d[:], ones_bf[:], acc[:],
                                         start=True, stop=True)
                        den_sb = spool.tile([1, 512], F32, name="den",
                                            tag="den")
                        nc.scalar.copy(den_sb[:], ps_d[:])
                        for u in range(4):
                            t0 = tj * 512 + u * P
                            ps_dt = pss.tile([P, 1], F32, name="ps_dt",
                                             tag="pss")
                            nc.tensor.transpose(
                                ps_dt[:], den_sb[:, u * P:(u + 1) * P],
                                id_f32[:1, :1],
                            )
                            rec = spool.tile([P, 1], F32, name="rec",
                                             tag="rec")
                            nc.vector.reciprocal(rec[:], ps_dt[:])
                            ps_yt = pss.tile([P, P], BF, name="ps_yt",
                                             tag="pss")
                            nc.tensor.transpose(
                                ps_yt[:], yT_sb[:, u * P:(u + 1) * P],
                                id_bf[:],
                            )
                            o_sb = opool.tile([P, HS], F32, name="o_sb",
                                              tag="o")
                            nc.scalar.activation(
                                o_sb[:], ps_yt[:], Copy, scale=rec[:]
                            )
                            nc.sync.dma_start(
                                out[h * T + t0: h * T + t0 + P, :], o_sb[:]
                            )

                    for h in range(HPC):
                        ps_y = {
                            tj: psy.tile([P, 512], F32, name=f"psy{h}_{tj}",
                                         tag="psy")
                            for tj in range(TJ)
                        }
                        acc = {
                            tj: accp.tile([P, 512], BF, name=f"acc{h}_{tj}",
                                          tag="acc")
                            for tj in range(TJ)
                        }
                        pend = {}

                        def emit_av(k):
                            # AV matmuls for chunk k (stationary v reused)
                            for tj, at_prev in pend.pop(k).items():
                                nc.tensor.matmul(
                                    ps_y[tj][:], vslice(k, h), at_prev[:],
                                    start=(k == 0), stop=(k == 4 * tj + 3),
                                )
                            # drain any (h, tj) whose last AV just ran
                            if k >= 3 and (k - 3) % 4 == 0:
                                tjd = (k - 3) // 4
                                tail(h, tjd, ps_y[tjd], acc[tjd])

                        for k in range(SC):
                            tjs = list(range(k // 4, TJ))
                            ats = {}
                            # sub-groups of <=3 so the 3-deep pss ring can't
                            # deadlock; stationary reused within each group
                            for gi in range(0, len(tjs), 3):
                                grp = tjs[gi:gi + 3]
                                ps_t = {}
                                for tj in grp:
                                    ps_s = pss.tile([P, 512], F32,
                                                    name="ps_s", tag="pss")
                                    nc.tensor.matmul(
                                        ps_s[:],
                                        kT[:, h * T + k * P:
                                           h * T + (k + 1) * P],
                                        qT[:, h * T + tj * 512:
                                           h * T + (tj + 1) * 512],
                                        start=True, stop=False,
                                    )
                                    ps_t[tj] = ps_s
                                for tj in grp:
                                    nc.tensor.matmul(
                                        ps_t[tj][:],
                                        kr_rope[:, k * P:(k + 1) * P],
                                        qr_rope[:, h * T + tj * 512:
                                                h * T + (tj + 1) * 512],
                                        start=False, stop=True,
                                    )
                                for tj in grp:
                                    at = atp.tile([P, 512], BF, name="at",
                                                  tag="at")
                                    nc.scalar.activation(
                                        at[:], ps_t[tj][:], Exp, scale=SCALE
                                    )
                                    if tj == k // 4:
                                        nc.vector.tensor_mul(
                                            at[:], at[:],
                                            mask01[:, (k % 4) * 512:
                                                   (k % 4 + 1) * 512],
                                        )
                                    if k == 0:
                                        nc.vector.tensor_copy(acc[tj][:],
                                                              at[:])
                                    else:
                                        nc.vector.tensor_add(
                                            acc[tj][:], acc[tj][:], at[:]
                                        )
                                    ats[tj] = at
                            pend[k] = ats
                            if k - 1 in pend:
                                emit_av(k - 1)
                        emit_av(SC - 1)
    nc.finalize()
    return nc


_ROPE_PERM = np.concatenate([np.arange(0, DHR, 2), np.arange(1, DHR, 2)])


def _bf(a):
    return np.ascontiguousarray(a).astype(ml_dtypes.bfloat16)


def _prep_inputs(x, freqs_cos, freqs_sin, W_dq, W_uq, W_dkv, W_uk, W_uv, W_qr,
                 W_kr, W_o):
    """Build the 8 per-core input maps (host-side layout prep, all bf16)."""
    x2 = np.asarray(x, np.float32).reshape(T, C)
    xT = np.ascontiguousarray(x2.T)                  # [C, T]
    xT_bf = _bf(xT).reshape(CCH, P, T)
    wdkvT = _bf(np.asarray(W_dkv).T.reshape(C, 1, 512).transpose(1, 0, 2))
    wkrT = _bf(np.asarray(W_kr)[_ROPE_PERM, :].T)    # [C, DHR], rope-permuted
    cosT = np.asarray(freqs_cos, np.float32).T       # [32, T]
    sinT = np.asarray(freqs_sin, np.float32).T
    cos2T = _bf(np.concatenate([cosT, cosT], axis=0))    # [64, T]
    sin2T = _bf(np.concatenate([-sinT, sinT], axis=0))

    Wdq = np.asarray(W_dq, np.float32)               # [NLQ, C]
    Wuq_mat = np.asarray(W_uq, np.float32).reshape(NLQ, NH * HS)
    Wq_comb = Wdq.T @ Wuq_mat                        # [C, NH*HS]
    Wqr_comb = Wdq.T @ np.asarray(W_qr, np.float32).T    # [C, NH*DHR]
    v_eff = np.asarray(W_uv, np.float32).T @ np.asarray(W_o, np.float32).T
    W_uk_a = np.asarray(W_uk)

    in_maps = []
    for i in range(NCORES):
        h0 = i * HPC
        cols = slice(h0 * HS, (h0 + HPC) * HS)       # 256 output cols
        wqr_cols = np.concatenate(
            [Wqr_comb[:, (h0 + h) * DHR + _ROPE_PERM] for h in range(HPC)],
            axis=1,
        )                                            # [C, HPC*64=128]
        in_maps.append({
            "xTp": xT_bf,
            "xs": _bf(xT[:, i * TS:(i + 1) * TS]),
            "wdkvT": wdkvT,
            "wkrT": wkrT,
            "cos2T": cos2T,
            "sin2T": sin2T,
            "wq": _bf(Wq_comb[:, cols]).reshape(CCH, P, HPC * HS),
            "wqr": _bf(wqr_cols).reshape(CCH, P, HPC * DHR),
            "wukT": _bf(np.ascontiguousarray(
                        W_uk_a[h0 * HS:(h0 + HPC) * HS, :].T)
                        .reshape(LKV, P, HPC * HS)),
            "bc": _bf(v_eff[:, cols]).reshape(LKV, P, HPC * HS),
        })
    return in_maps


_NC_CACHE = None


def kernel(**inputs):
    global _NC_CACHE
    in_maps = _prep_inputs(**inputs)
    if _NC_CACHE is None:
        _NC_CACHE = build_nc()
    res = run_bass_kernel_spmd(_NC_CACHE, in_maps, core_ids=list(range(NCORES)))
    outs = [np.asarray(res.results[i]["out"], np.float32)
            .reshape(HPC, T, HS).transpose(1, 0, 2).reshape(T, HPC * HS)
            for i in range(NCORES)]
    y = np.concatenate(outs, axis=1).reshape(B, T, C)
    return y


# revision 22
# speedup vs baseline: 1.5764x; 1.0701x over previous
"""MLA-style attention (nn_Attention_7868380086611) on 8 TRN2 NeuronCores.

Strategy (v3)
-------------
Head-parallel attention (2 of 16 heads per core).  The query path is fully
absorbed on the host into per-head combined weights (W_dq.T @ W_uq and
W_dq.T @ W_qr.T — weight-only products, same trick as the reference's own
v_eff absorption), so each core computes q/q_r for its 2 heads directly
from the full x with NO collective.  Only the tiny shared kv latent
(c_kv: 512 rows, k_r: 64 rows per token) is computed T-sharded and
AllGathered once (~288 KB per rank); the gather is overlapped with the
q-projection matmuls.  v_eff = W_uv.T @ W_o.T is host-precomputed.

v3 kernel-side improvements over v2:
- PE warm-up matmuls at t=0 so the HAM clock gate opens (2.4 GHz) before
  the real work starts.
- Attention is k-outer with software-pipelined AV matmuls (one k-chunk
  behind the score matmuls) so the tensor queue never stalls on exp; the
  stationary operand (kT/kr/v slice) is reused across the tj blocks of
  one k-chunk, cutting LDWEIGHTS count ~2.5x.
- Causal mask is a multiplicative bf16 0/1 mask applied to exp() output
  (vector 2x mode) instead of a -1e10 f32 add into PSUM (1x mode).
- Softmax denominator accumulates in bf16 (vector 2x) and one
  ones-matmul per (head, tj) on the bf16 accumulator.
- Projection loops are tj-inner so the stationary weight tile is reused
  across 4 matmuls (4x fewer LDWEIGHTS).
"""

import math
import sys

import numpy as np

sys.path.insert(0, "/opt/trn_rl_repo")

import ml_dtypes  # noqa: E402

from concourse import bacc, bass, masks, mybir  # noqa: E402
from concourse.bass_utils import run_bass_kernel_spmd  # noqa: E402
from concourse.tile import TileContext  # noqa: E402

B, T, C = 1, 2048, 2048
NH, HS = 16, 128
NLQ, NLKV, DHR = 1536, 512, 64
NCORES = 8
HPC = NH // NCORES          # heads per core = 2
TS = T // NCORES            # 256-token shard for the kv down-projection
P = 128
LKV = NLKV // P             # 4
CCH = C // P                # 16 c-chunks
TJ = T // 512               # 4 t-chunks of 512
SC = T // P                 # 16 s-chunks
SCALE = 1.0 / math.sqrt(HS + DHR)

BF = mybir.dt.bfloat16
F32 = mybir.dt.float32
Exp = mybir.ActivationFunctionType.Exp
Copy = mybir.ActivationFunctionType.Copy

GKV = NLKV + DHR            # 576 rows in the all-gather buffer


def build_nc():
    nc = bacc.Bacc(None, target_bir_lowering=False, num_devices=NCORES)

    xTp = nc.declare_dram_parameter("xTp", [CCH, P, T], BF, isOutput=False)
    wdkvT = nc.declare_dram_parameter("wdkvT", [1, C, 512], BF, isOutput=False)
    wkrT = nc.declare_dram_parameter("wkrT", [C, DHR], BF, isOutput=False)
    cos2T = nc.declare_dram_parameter("cos2T", [DHR, T], BF, isOutput=False)
    sin2T = nc.declare_dram_parameter("sin2T", [DHR, T], BF, isOutput=False)
    wq = nc.declare_dram_parameter("wq", [CCH, P, HPC * HS], BF, isOutput=False)
    wqr = nc.declare_dram_parameter("wqr", [CCH, P, HPC * DHR], BF, isOutput=False)
    wukT = nc.declare_dram_parameter("wukT", [LKV, P, HPC * HS], BF, isOutput=False)
    bc = nc.declare_dram_parameter("bc", [LKV, P, HPC * HS], BF, isOutput=False)
    xs = nc.declare_dram_parameter("xs", [C, TS], BF, isOutput=False)
    out = nc.declare_dram_parameter("out", [HPC * T, HS], F32, isOutput=True)

    cc_in_kv = nc.dram_tensor("cc_in_kv", [GKV, TS], BF)
    cc_out_kv = nc.dram_tensor("cc_out_kv", [NCORES, GKV, TS], BF,
                               addr_space="Shared")

    with TileContext(nc) as tc:
        with (
            tc.tile_pool(name="persist", bufs=1) as persist,
            tc.tile_pool(name="lat", bufs=1) as lat,
            tc.tile_pool(name="proj", bufs=1) as proj,
            tc.tile_pool(name="wts", bufs=1) as wts,
        ):
            # ---- constants ----
            id_bf = persist.tile([P, P], BF)
            masks.make_identity(nc, id_bf[:])
            id_f32 = persist.tile([P, P], F32)
            masks.make_identity(nc, id_f32[:])
            ones_bf = persist.tile([P, 1], BF)
            nc.vector.memset(ones_bf[:], 1.0)
            # 4 multiplicative causal masks [128, 512]: 1 iff t - s - 128*m >= 0
            mask01 = persist.tile([P, 4 * 512], BF)
            nc.vector.memset(mask01[:], 1.0)
            for m in range(4):
                nc.gpsimd.affine_select(
                    out=mask01[:, m * 512:(m + 1) * 512],
                    in_=mask01[:, m * 512:(m + 1) * 512],
                    compare_op=mybir.AluOpType.is_ge,
                    fill=0.0,
                    base=-m * P,
                    channel_multiplier=-1,
                    pattern=[[1, 512]],
                )
            # cos/sin tiles: loaded on the scalar queue but only AFTER the
            # phase-1 bounce stores (emitted below) so they don't delay the
            # AllGather trigger
            cos_sb = persist.tile([DHR, T], BF)
            sin_sb = persist.tile([DHR, T], BF)

            # ---- phase 1: c_kv^T/k_r^T for own T/8 slice -> AllGather.
            # The rank-dependent x column slice comes in pre-sliced (xs) so
            # the SPMD graph stays rank-independent.
            with (
                tc.tile_pool(name="p1w", bufs=1) as p1w,
                tc.tile_pool(name="p1ps", bufs=2, space="PSUM") as p1ps,
                tc.tile_pool(name="p1sh", bufs=3) as p1sh,
                tc.tile_pool(name="p1xs", bufs=1) as p1xs,
            ):
                # PE warm-up: N=512 throwaway matmuls bridging until the xs
                # DMA lands (~9us) so the HAM clock gate opens (2.4 GHz) and
                # STAYS open into the first real accumulation chain.
                junk = p1w.tile([P, 512], BF, name="junk")
                nc.vector.memset(junk[:], 0.0)
                ps_w = p1ps.tile([P, 512], F32, name="ps_warm", tag="warm")
                for _ in range(16):
                    nc.tensor.matmul(ps_w[:], id_bf[:], junk[:],
                                     start=True, stop=True)

                xsl = []
                for cgrp in range(4):
                    tsl = p1xs.tile([P, 4 * TS], BF, name=f"xsl{cgrp}",
                                    tag=f"xsl{cgrp}")
                    nc.sync.dma_start(
                        tsl[:].rearrange("p (n u) -> p n u", n=4),
                        xs.ap().rearrange("(n p) u -> n p u", p=P)
                        [4 * cgrp:4 * (cgrp + 1)].rearrange("n p u -> p n u"),
                    )
                    xsl.append(tsl)

                def xstile(c):
                    return xsl[c // 4][:, (c % 4) * TS:(c % 4 + 1) * TS]

                # wdkv in 4 group tiles so the first chain starts early
                w4 = []
                for g in range(4):
                    wt = p1w.tile([P, 4 * 512], BF, name=f"wdkv{g}",
                                  tag=f"wdkv{g}")
                    nc.sync.dma_start(
                        wt[:].rearrange("p (n m) -> p n m", n=4),
                        wdkvT[0].rearrange("(n p) m -> p n m", p=P)
                        [:, 4 * g:4 * (g + 1), :].rearrange("p n m -> p n m"),
                    )
                    w4.append(wt)
                wkr_sb = p1w.tile([P, CCH * DHR], BF, name="wkr_sb")
                nc.sync.dma_start(
                    wkr_sb[:].rearrange("p (n m) -> p n m", n=CCH),
                    wkrT.ap().rearrange("(n p) m -> p n m", p=P),
                )

                def wdkv_sl(c, ls):
                    return w4[c // 4][:, (c % 4) * 512 + ls * P:
                                      (c % 4) * 512 + (ls + 1) * P]

                for ls in range(LKV):
                    ps = p1ps.tile([P, TS], F32, name="p1ps_t", tag="p1ps_t")
                    for c in range(CCH):
                        nc.tensor.matmul(
                            ps[:], wdkv_sl(c, ls), xstile(c),
                            start=(c == 0), stop=(c == CCH - 1),
                        )
                    sh = p1sh.tile([P, TS], BF, name="p1sh_t", tag="p1sh_t")
                    nc.scalar.copy(sh[:], ps[:])
                    nc.scalar.dma_start(
                        cc_in_kv[ls * P:(ls + 1) * P, :], sh[:]
                    )
                ps_kr = p1ps.tile([DHR, TS], F32, name="ps_kr", tag="p1ps_t")
                for c in range(CCH):
                    nc.tensor.matmul(
                        ps_kr[:],
                        wkr_sb[:, c * DHR:(c + 1) * DHR],
                        xstile(c),
                        start=(c == 0),
                        stop=(c == CCH - 1),
                    )
                sh_kr = p1sh.tile([DHR, TS], BF, name="sh_kr")
                nc.scalar.copy(sh_kr[:], ps_kr[:])
                nc.scalar.dma_start(cc_in_kv[NLKV:GKV, :], sh_kr[:])

                nc.gpsimd.collective_compute(
                    "AllGather",
                    mybir.AluOpType.bypass,
                    replica_groups=[list(range(NCORES))],
                    ins=[cc_in_kv.ap().opt()],
                    outs=[cc_out_kv.ap().opt()],
                )

            nc.scalar.dma_start(cos_sb[:], cos2T[:, :])
            nc.scalar.dma_start(sin_sb[:], sin2T[:, :])

            # ---- projection weights, then full x^T (sync-queue order) ----
            wq_all = wts.tile([P, CCH * HPC * HS], BF)
            nc.sync.dma_start(
                wq_all[:].rearrange("p (n m) -> p n m", n=CCH),
                wq.ap().rearrange("n p m -> p n m"),
            )
            wqr_all = wts.tile([P, CCH * HPC * DHR], BF)
            nc.sync.dma_start(
                wqr_all[:].rearrange("p (n m) -> p n m", n=CCH),
                wqr.ap().rearrange("n p m -> p n m"),
            )
            # x^T chunk loads: 16 simple [128, T] DMAs — cheap descriptor
            # generation vs the rearranged group loads (0.65us vs up to 6us
            # of sync-sequencer time each)
            xt = []
            for cgrp in range(4):
                t = lat.tile([P, 4 * T], BF, name=f"xt{cgrp}", tag=f"xt{cgrp}")
                for j in range(4):
                    nc.sync.dma_start(
                        t[:, j * T:(j + 1) * T], xTp.ap()[4 * cgrp + j]
                    )
                xt.append(t)

            def xtile(c):
                return xt[c // 4][:, (c % 4) * T:(c % 4 + 1) * T]

            wuk_all = wts.tile([P, LKV * HPC * HS], BF)
            nc.sync.dma_start(
                wuk_all[:].rearrange("p (n m) -> p n m", n=LKV),
                wukT.ap().rearrange("n p m -> p n m"),
            )
            b_all = wts.tile([P, LKV * HPC * HS], BF)
            nc.sync.dma_start(
                b_all[:].rearrange("p (n m) -> p n m", n=LKV),
                bc.ap().rearrange("n p m -> p n m"),
            )

            with tc.tile_pool(name="rtmp", bufs=1) as rtmp:

                def rope(dst, src):
                    # dst = src * [cos;cos] + swap_halves(src) * [-sin;sin]
                    sw = rtmp.tile([DHR, T], BF, name="rsw", tag="rsw")
                    nc.sync.dma_start(sw[0:32, :], src[32:64, :])
                    nc.sync.dma_start(sw[32:64, :], src[0:32, :])
                    ta = rtmp.tile([DHR, T], BF, name="rta", tag="rta")
                    tb = rtmp.tile([DHR, T], BF, name="rtb", tag="rtb")
                    nc.vector.tensor_mul(ta[:], src, cos_sb[:])
                    nc.vector.tensor_mul(tb[:], sw[:], sin_sb[:])
                    nc.vector.tensor_add(dst, ta[:], tb[:])

                qT = proj.tile([P, HPC * T], BF)
                kT = proj.tile([P, HPC * T], BF)
                qr_rope = proj.tile([DHR, HPC * T], BF)
                qr2 = proj.tile([P, T], BF)          # merged 2-head qr, pre-split
                qr_h1 = proj.tile([DHR, T], BF)      # head-1 rows on part 0-63
                v_sb = proj.tile([P, SC * HPC * HS], BF)
                kr_rope = proj.tile([DHR, T], BF)

                with tc.tile_pool(name="p5ps", bufs=5, space="PSUM") as p5ps:
                    # q_r^T both heads in one pass (M=128), tj-inner so the
                    # stationary wqr chunk is loaded once per c
                    ps_qr = [
                        p5ps.tile([P, 512], F32, name=f"ps_qr{tj}", tag="p5")
                        for tj in range(TJ)
                    ]
                    for c in range(CCH):
                        for tj in range(TJ):
                            nc.tensor.matmul(
                                ps_qr[tj][:],
                                wqr_all[:, c * HPC * DHR:(c + 1) * HPC * DHR],
                                xtile(c)[:, tj * 512:(tj + 1) * 512],
                                start=(c == 0),
                                stop=(c == CCH - 1),
                            )
                    for tj in range(TJ):
                        nc.vector.tensor_copy(
                            qr2[:, tj * 512:(tj + 1) * 512], ps_qr[tj][:]
                        )
                    nc.sync.dma_start(qr_h1[:, :], qr2[DHR:P, :])
                    rope(qr_rope[:, 0:T], qr2[0:DHR, :])
                    rope(qr_rope[:, T:HPC * T], qr_h1[:, :])

                    # q^T per head, tj-inner
                    for h in range(HPC):
                        ps_q = [
                            p5ps.tile([P, 512], F32, name=f"ps_q{h}_{tj}",
                                      tag="p5")
                            for tj in range(TJ)
                        ]
                        for c in range(CCH):
                            for tj in range(TJ):
                                nc.tensor.matmul(
                                    ps_q[tj][:],
                                    wq_all[:, c * HPC * HS + h * HS:
                                           c * HPC * HS + (h + 1) * HS],
                                    xtile(c)[:, tj * 512:(tj + 1) * 512],
                                    start=(c == 0),
                                    stop=(c == CCH - 1),
                                )
                        for tj in range(TJ):
                            nc.scalar.copy(
                                qT[:, h * T + tj * 512: h * T + (tj + 1) * 512],
                                ps_q[tj][:],
                            )

                    # ---- gathered kv latents ----
                    ckv_t = []
                    for l in range(LKV):
                        t = lat.tile([P, T], BF, name=f"ckv{l}", tag=f"ckv{l}")
                        nc.sync.dma_start(
                            t[:].rearrange("p (g u) -> p g u", g=NCORES),
                            cc_out_kv[:, l * P:(l + 1) * P, :].rearrange(
                                "g p u -> p g u"
                            ),
                        )
                        ckv_t.append(t)
                    kr_raw = lat.tile([DHR, T], BF)
                    nc.sync.dma_start(
                        kr_raw[:].rearrange("p (g u) -> p g u", g=NCORES),
                        cc_out_kv[:, NLKV:GKV, :].rearrange("g p u -> p g u"),
                    )
                    rope(kr_rope[:, :], kr_raw[:, :])

                    # k^T per head, sj-inner (stationary wuk chunk reused)
                    for h in range(HPC):
                        ps_k = [
                            p5ps.tile([P, 512], F32, name=f"ps_k{h}_{sj}",
                                      tag="p5")
                            for sj in range(TJ)
                        ]
                        for l in range(LKV):
                            for sj in range(TJ):
                                nc.tensor.matmul(
                                    ps_k[sj][:],
                                    wuk_all[:, l * HPC * HS + h * HS:
                                            l * HPC * HS + (h + 1) * HS],
                                    ckv_t[l][:, sj * 512:(sj + 1) * 512],
                                    start=(l == 0),
                                    stop=(l == LKV - 1),
                                )
                        for sj in range(TJ):
                            nc.scalar.copy(
                                kT[:, h * T + sj * 512: h * T + (sj + 1) * 512],
                                ps_k[sj][:],
                            )
                    # v~ per s-chunk
                    for sc in range(SC):
                        ps = p5ps.tile([P, HPC * HS], F32, name="ps_v",
                                       tag="p5v", bufs=3)
                        for l in range(LKV):
                            nc.tensor.matmul(
                                ps[:],
                                ckv_t[l][:, sc * P:(sc + 1) * P],
                                b_all[:, l * HPC * HS:(l + 1) * HPC * HS],
                                start=(l == 0),
                                stop=(l == LKV - 1),
                            )
                        nc.vector.tensor_copy(
                            v_sb[:, sc * HPC * HS:(sc + 1) * HPC * HS], ps[:]
                        )

                # ---- attention (causal, k-outer, AV pipelined one k behind).
                with (
                    tc.tile_pool(name="psy", bufs=4, space="PSUM") as psy,
                    tc.tile_pool(name="pss", bufs=4, space="PSUM") as pss,
                    tc.tile_pool(name="atp", bufs=9) as atp,
                    tc.tile_pool(name="accp", bufs=6) as accp,
                    tc.tile_pool(name="spool", bufs=3) as spool,
                    tc.tile_pool(name="opool", bufs=3) as opool,
                ):
                    def vslice(k, h):
                        return v_sb[:, k * HPC * HS + h * HS:
                                    k * HPC * HS + (h + 1) * HS]

                    def tail(h, tj, ps_y, acc):
                        yT_sb = spool.tile([P, 512], BF, name="yT", tag="yT")
                        nc.scalar.copy(yT_sb[:], ps_y[:])
                        ps_d = pss.tile([1, 512], F32, name="ps_d", tag="pss")
                        nc.tensor.matmul(ps_d[:], ones_bf[:], acc[:],
                                         start=True, stop=True)
                        den_sb = spool.tile([1, 512], F32, name="den",
                                            tag="den")
                        nc.scalar.copy(den_sb[:], ps_d[:])
                        for u in range(4):
                            t0 = tj * 512 + u * P
                            ps_dt = pss.tile([P, 1], F32, name="ps_dt",
                                             tag="pss")
                            nc.tensor.transpose(
                                ps_dt[:], den_sb[:, u * P:(u + 1) * P],
                                id_f32[:1, :1],
                            )
                            rec = spool.tile([P, 1], F32, name="rec",
                                             tag="rec")
                            nc.vector.reciprocal(rec[:], ps_dt[:])
                            ps_yt = pss.tile([P, P], BF, name="ps_yt",
                                             tag="pss")
                            nc.tensor.transpose(
                                ps_yt[:], yT_sb[:, u * P:(u + 1) * P],
                                id_bf[:],
                            )
                            o_sb = opool.tile([P, HS], F32, name="o_sb",
                                              tag="o")
                            nc.scalar.activation(
                                o_sb[:], ps_yt[:], Copy, scale=rec[:]
                            )
                            nc.sync.dma_start(
                                out[h * T + t0: h * T + t0 + P, :], o_sb[:]
                            )

                    for h in range(HPC):
                        ps_y = {
                            tj: psy.tile([P, 512], F32, name=f"psy{h}_{tj}",
                                         tag="psy")
                            for tj in range(TJ)
                        }
                        acc = {
                            tj: accp.tile([P, 512], BF, name=f"acc{h}_{tj}",
                                          tag="acc")
                            for tj in range(TJ)
                        }
                        pend = {}

                        def emit_av(k):
                            # AV matmuls for chunk k (stationary v reused)
                            for tj, at_prev in pend.pop(k).items():
                                nc.tensor.matmul(
                                    ps_y[tj][:], vslice(k, h), at_prev[:],
                                    start=(k == 0), stop=(k == 4 * tj + 3),
                                )
                            # drain any (h, tj) whose last AV just ran
                            if k >= 3 and (k - 3) % 4 == 0:
                                tjd = (k - 3) // 4
                                tail(h, tjd, ps_y[tjd], acc[tjd])

                        for k in range(SC):
                            tjs = list(range(k // 4, TJ))
                            ats = {}
                            # sub-groups of <=3 so the 3-deep pss ring can't
                            # deadlock; stationary reused within each group
                            for gi in range(0, len(tjs), 3):
                                grp = tjs[gi:gi + 3]
                                ps_t = {}
                                for tj in grp:
                                    ps_s = pss.tile([P, 512], F32,
                                                    name="ps_s", tag="pss")
                                    nc.tensor.matmul(
                                        ps_s[:],
                                        kT[:, h * T + k * P:
                                           h * T + (k + 1) * P],
                                        qT[:, h * T + tj * 512:
                                           h * T + (tj + 1) * 512],
                                        start=True, stop=False,
                                    )
                                    ps_t[tj] = ps_s
                                for tj in grp:
                                    nc.tensor.matmul(
                                        ps_t[tj][:],
                                        kr_rope[:, k * P:(k + 1) * P],
                                        qr_rope[:, h * T + tj * 512:
                                                h * T + (tj + 1) * 512],
                                        start=False, stop=True,
                                    )
                                for tj in grp:
                                    at = atp.tile([P, 512], BF, name="at",
                                                  tag="at")
                                    nc.scalar.activation(
                                        at[:], ps_t[tj][:], Exp, scale=SCALE
                                    )
                                    if tj == k // 4:
                                        nc.vector.tensor_mul(
                                            at[:], at[:],
                                            mask01[:, (k % 4) * 512:
                                                   (k % 4 + 1) * 512],
                                        )
                                    if k == 0:
                                        nc.vector.tensor_copy(acc[tj][:],
                                                              at[:])
                                    else:
                                        nc.vector.tensor_add(
                                            acc[tj][:], acc[tj][:], at[:]
                                        )
                                    ats[tj] = at
                            pend[k] = ats
                            if k - 1 in pend:
                                emit_av(k - 1)
                        emit_av(SC - 1)
    nc.finalize()
    return nc


_ROPE_PERM = np.concatenate([np.arange(0, DHR, 2), np.arange(1, DHR, 2)])


def _bf(a):
    return np.ascontiguousarray(a).astype(ml_dtypes.bfloat16)


def _prep_inputs(x, freqs_cos, freqs_sin, W_dq, W_uq, W_dkv, W_uk, W_uv, W_qr,
                 W_kr, W_o):
    """Build the 8 per-core input maps (host-side layout prep, all bf16)."""
    x2 = np.asarray(x, np.float32).reshape(T, C)
    xT = np.ascontiguousarray(x2.T)                  # [C, T]
    xT_bf = _bf(xT).reshape(CCH, P, T)
    wdkvT = _bf(np.asarray(W_dkv).T.reshape(C, 1, 512).transpose(1, 0, 2))
    wkrT = _bf(np.asarray(W_kr)[_ROPE_PERM, :].T)    # [C, DHR], rope-permuted
    cosT = np.asarray(freqs_cos, np.float32).T       # [32, T]
    sinT = np.asarray(freqs_sin, np.float32).T
    cos2T = _bf(np.concatenate([cosT, cosT], axis=0))    # [64, T]
    sin2T = _bf(np.concatenate([-sinT, sinT], axis=0))

    Wdq = np.asarray(W_dq, np.float32)               # [NLQ, C]
    Wuq_mat = np.asarray(W_uq, np.float32).reshape(NLQ, NH * HS)
    Wq_comb = Wdq.T @ Wuq_mat                        # [C, NH*HS]
    Wqr_comb = Wdq.T @ np.asarray(W_qr, np.float32).T    # [C, NH*DHR]
    v_eff = np.asarray(W_uv, np.float32).T @ np.asarray(W_o, np.float32).T
    W_uk_a = np.asarray(W_uk)

    in_maps = []
    for i in range(NCORES):
        h0 = i * HPC
        cols = slice(h0 * HS, (h0 + HPC) * HS)       # 256 output cols
        wqr_cols = np.concatenate(
            [Wqr_comb[:, (h0 + h) * DHR + _ROPE_PERM] for h in range(HPC)],
            axis=1,
        )                                            # [C, HPC*64=128]
        in_maps.append({
            "xTp": xT_bf,
            "xs": _bf(xT[:, i * TS:(i + 1) * TS]),
            "wdkvT": wdkvT,
            "wkrT": wkrT,
            "cos2T": cos2T,
            "sin2T": sin2T,
            "wq": _bf(Wq_comb[:, cols]).reshape(CCH, P, HPC * HS),
            "wqr": _bf(wqr_cols).reshape(CCH, P, HPC * DHR),
            "wukT": _bf(np.ascontiguousarray(
                        W_uk_a[h0 * HS:(h0 + HPC) * HS, :].T)
                        .reshape(LKV, P, HPC * HS)),
            "bc": _bf(v_eff[:, cols]).reshape(LKV, P, HPC * HS),
        })
    return in_maps


_NC_CACHE = None


def kernel(**inputs):
    global _NC_CACHE
    in_maps = _prep_inputs(**inputs)
    if _NC_CACHE is None:
        _NC_CACHE = build_nc()
    res = run_bass_kernel_spmd(_NC_CACHE, in_maps, core_ids=list(range(NCORES)))
    outs = [np.asarray(res.results[i]["out"], np.float32)
            .reshape(HPC, T, HS).transpose(1, 0, 2).reshape(T, HPC * HS)
            for i in range(NCORES)]
    y = np.concatenate(outs, axis=1).reshape(B, T, C)
    return y
